# revision 12
# baseline (speedup 1.0000x reference)
"""Trainium2 Bass kernel for nn_MATAPCell (GRU + single-query MHA over per-row
memory + gated blend + memory shift-write).

Contract: kernel(**inputs) takes FULL unsharded fp32 inputs (see shapes below),
shards batch across 8 NeuronCores (pure data parallel, weights replicated),
runs a Bass/Tile kernel per core, and gathers the full outputs.

Returns (h_corr [B,256] f32, new_memory_flat [B,2560] f32) matching reference.

Dataflow per core (BS=1024 rows, two column-groups of 512):
  - activations live TRANSPOSED in SBUF: [feature(partitions), row(free)]
  - natural-layout rows are DMA'd in and PE-transposed (f32) into bf16 tiles
  - all matmuls bf16 (weights host-cast), fp32 PSUM accumulate
  - attention: k = mem@Wk on PE; scores q.k via DVE mult + PE mask-matmul
    partition reduction; softmax (no max-subtract: |scores| < 0.3);
    apply: v on PE, attn replicated across dk-partitions via PE mask-matmul,
    DVE mult, wide-AP tree accumulation
  - h-path carrier (h_prev -> gru_out -> blend -> LN2 -> h_corr) kept f32
    in elementwise ops for accuracy
  - memory shift is a pure SBUF->DRAM f32 copy of the loaded mem tile
  - layernorms computed in T-layout via ones-matmul partition sums + rank-1
    replication matmuls
"""

import sys
import os
import numpy as np

for _p in ("/opt/trn_rl_repo",):
    if os.path.isdir(_p) and _p not in sys.path:
        sys.path.insert(0, _p)

import ml_dtypes
from contextlib import ExitStack

import concourse.bass as bass
import concourse.tile as tile
from concourse import bacc, mybir
from concourse.bass_utils import run_bass_kernel_spmd

BF16 = ml_dtypes.bfloat16
FP32 = mybir.dt.float32
BF = mybir.dt.bfloat16

B, D, M, H, DK = 8192, 256, 10, 4, 64
N_CORES = 8
BS = B // N_CORES            # 1024 rows per core
NG = BS // 512               # column groups per core (512 cols each)
GC = 512                     # cols per group
NR4 = 4                      # row-tiles (128 rows) per group
HKD = H * DK                 # 256
D3 = 3 * D                   # 768
LN_EPS = 1e-3
ALU = mybir.AluOpType
ACTF = mybir.ActivationFunctionType

_BUILD_CACHE = {}


def _mask_tensors():
    """Host-precomputed bf16 mask/constant matmul operands."""
    # score mask: lhsT [128, 40]; partition p of hk-tile t belongs to global
    # head h = 2*t + p//64; column index (h*10 + m) gets 1.
    smask = np.zeros((M, 2, 128, H * M), np.float32)
    for m in range(M):
        for t in range(2):
            for p in range(128):
                h = 2 * t + p // DK
                smask[m, t, p, h * M + m] = 1.0
    # apply mask: lhsT [40, 128]; out partition p of hk-tile t reads attn row
    # (h*10+m) with h = 2*t + p//64.
    amask = np.zeros((M, 2, H * M, 128), np.float32)
    for m in range(M):
        for t in range(2):
            for p in range(128):
                h = 2 * t + p // DK
                amask[m, t, h * M + m, p] = 1.0
    # sum mask [40, 4]: row (h*10+m) -> col h
    summask = np.zeros((H * M, H), np.float32)
    for h in range(H):
        summask[h * M:(h + 1) * M, h] = 1.0
    # replicate [4, 40]: row h -> cols h*10..h*10+9
    repl = np.zeros((H, H * M), np.float32)
    for h in range(H):
        repl[h, h * M:(h + 1) * M] = 1.0
    ones_128_1 = np.ones((128, 1), np.float32)
    ones_1_128 = np.ones((1, 128), np.float32)
    ident = np.eye(128, dtype=np.float32)
    return {
        "smask": smask.astype(BF16),
        "amask": amask.astype(BF16),
        "summask": summask.astype(BF16),
        "replmask": repl.astype(BF16),
        "ones_128_1": ones_128_1.astype(BF16),
        "ones_1_128": ones_1_128.astype(BF16),
        "ident_f32": ident,
    }


def _prep_weights(inp):
    """Host-side weight fusion + bf16 casts. All small (<3MB)."""
    f = lambda x: np.asarray(x, np.float32)
    W_in = f(inp["W_in"]); b_in = f(inp["b_in"])
    gru_k = f(inp["gru_k"]); gru_rk = f(inp["gru_rk"]); gru_b = f(inp["gru_b"])
    Wq = f(inp["Wq"]).reshape(D, HKD); bq = f(inp["bq"]).reshape(HKD)
    Wk = f(inp["Wk"]).reshape(D, HKD)
    Wv = f(inp["Wv"]).reshape(D, HKD); bv = f(inp["bv"]).reshape(HKD)
    Wo = f(inp["Wo"]).reshape(HKD, D); bo = f(inp["bo"])
    g_attn = f(inp["g_attn"]); beta_attn = f(inp["beta_attn"])
    g_out = f(inp["g_out"]); beta_out = f(inp["beta_out"])
    W_ctx = f(inp["W_ctx"]); b_ctx = f(inp["b_ctx"])
    W_gate = f(inp["W_gate"]); b_gate = f(inp["b_gate"])
    W_mem = f(inp["W_mem"]); b_mem = f(inp["b_mem"])

    w = {}
    bf = lambda x: np.ascontiguousarray(x, dtype=np.float32).astype(BF16)
    # x-chain fused through W_in
    w["w_zr_x"] = bf(W_in @ gru_k[:, :2 * D])          # [256, 512]
    w["w_h_x"] = bf(W_in @ gru_k[:, 2 * D:])           # [256, 256]
    w["w_zr_h"] = bf(gru_rk[:, :2 * D])                # [256, 512]
    w["w_hh"] = bf(gru_rk[:, 2 * D:])                  # [256, 256]
    w["w_ne"] = bf(W_in @ W_mem)                       # [256, 256]
    w["w_q"] = bf(Wq)
    w["w_k"] = bf(Wk)
    w["w_v"] = bf(Wv)
    w["w_o"] = bf(Wo)
    # LN1 fused into W_ctx: Wc' = diag(g_attn) @ W_ctx
    w["w_ctx"] = bf(g_attn[:, None] * W_ctx)           # [256, 256]
    w["w_gate"] = bf(W_gate)                           # [512, 256]
    # LN1 rank-1 subtract weights: -(g_attn @ W_ctx) scaled later by mu (=S1/256)
    gw = g_attn @ W_ctx                                # [256]
    w["neg_gw"] = bf(-(gw / np.float32(D))[None, :])   # [1, 256]; rhs will be S1
    # LN2 rank-1: A*g replication and (B*g - beta)
    w["g_out_row"] = bf(g_out[None, :])                # [1, 256]
    w["neg_beta_out_row"] = bf(-beta_out[None, :])     # [1, 256]
    w["ones_row"] = bf(np.ones((1, 512), np.float32))  # [1, GC]

    # fp32 per-partition bias vectors, packed [256, nvec]
    b_zr = gru_b[0, :2 * D] + gru_b[1, :2 * D] + b_in @ gru_k[:, :2 * D]   # [512]
    b_xh = gru_b[0, 2 * D:] + b_in @ gru_k[:, 2 * D:]                      # [256]
    b_hh = gru_b[1, 2 * D:]                                                # [256]
    bq_s = bq / np.float32(np.sqrt(DK))                                    # [256]
    bo_p = bv @ Wo + bo                                                    # [256]
    # LN1-fused tanh bias: beta_attn @ W_ctx + b_ctx
    bw_ctx = beta_attn @ W_ctx + b_ctx                                     # [256]
    b_ne = b_in @ W_mem + b_mem                                            # [256]
    eps_col = np.full((D,), np.float32(D) * np.float32(D) * np.float32(LN_EPS), np.float32)
    vec_cols = [b_xh, b_hh, bq_s, bo_p, bw_ctx, b_gate, b_ne, eps_col]
    w["vecs"] = np.ascontiguousarray(np.stack(vec_cols, axis=1), np.float32)  # [256, 7]
    w["b_zr"] = np.ascontiguousarray(b_zr[:, None], np.float32)               # [512, 1]
    w.update(_mask_tensors())
    return w


VEC = {"b_xh": 0, "b_hh": 1, "bq_s": 2, "bo_p": 3, "bw_ctx": 4, "b_gate": 5,
       "b_ne": 6, "eps_dd": 7}
NVEC = 8


def build_kernel():
    nc = bacc.Bacc("TRN2", target_bir_lowering=False, debug=False,
                   num_devices=N_CORES)

    dram = {}

    def din(name, shape, dt=FP32):
        dram[name] = nc.dram_tensor(name, shape, dt, kind="ExternalInput").ap()
        return dram[name]

    def dout(name, shape, dt=FP32):
        dram[name] = nc.dram_tensor(name, shape, dt, kind="ExternalOutput").ap()
        return dram[name]

    x_d = din("x", (BS, D))
    h_d = din("h", (BS, D))
    mem_d = din("mem", (BS, M * D))
    w_zr_x = din("w_zr_x", (D, 2 * D), BF)
    w_zr_h = din("w_zr_h", (D, 2 * D), BF)
    w_h_x = din("w_h_x", (D, D), BF)
    w_hh = din("w_hh", (D, D), BF)
    w_ne = din("w_ne", (D, D), BF)
    w_q = din("w_q", (D, HKD), BF)
    w_k = din("w_k", (D, HKD), BF)
    w_v = din("w_v", (D, HKD), BF)
    w_o = din("w_o", (HKD, D), BF)
    w_ctx = din("w_ctx", (D, D), BF)
    w_gate = din("w_gate", (2 * D, D), BF)
    neg_gw = din("neg_gw", (1, D), BF)
    g_out_row = din("g_out_row", (1, D), BF)
    neg_beta_out_row = din("neg_beta_out_row", (1, D), BF)
    vecs_d = din("vecs", (D, NVEC))
    b_zr_d = din("b_zr", (2 * D, 1))
    ones_row_d = din("ones_row", (1, GC), BF)
    smask_d = din("smask", (M, 2, 128, H * M), BF)
    amask_d = din("amask", (M, 2, H * M, 128), BF)
    summask_d = din("summask", (H * M, H), BF)
    replmask_d = din("replmask", (H, H * M), BF)
    ones_128_1_d = din("ones_128_1", (128, 1), BF)
    ones_1_128_d = din("ones_1_128", (1, 128), BF)
    ident_d = din("ident_f32", (128, 128))

    hc_d = dout("hcorr", (BS, D))
    nm_d = dout("newmem", (BS, M * D))

    with tile.TileContext(nc) as tc, ExitStack() as ctx:
        konst = ctx.enter_context(tc.tile_pool(name="konst", bufs=1))
        nat = ctx.enter_context(tc.tile_pool(name="nat", bufs=1))
        big = ctx.enter_context(tc.tile_pool(name="big", bufs=1))
        act = ctx.enter_context(tc.tile_pool(name="act", bufs=1))
        sml = ctx.enter_context(tc.tile_pool(name="sml", bufs=1))
        pst = ctx.enter_context(tc.tile_pool(name="pst", bufs=1, space="PSUM"))

        PS_BUFS = 7

        def ps_tile(name, shape=None):
            return pst.tile(shape or [128, GC], FP32, tag="ps", bufs=PS_BUFS,
                            name=name)

        # ---- load constants ----
        def kload(ap_d, shape, dt, name):
            t = konst.tile(shape, dt, tag=name, name=name)
            nc.sync.dma_start(t[:, :], ap_d)
            return t

        W = {}
        for nm, ap_d, shape in [
            ("w_zr_x", w_zr_x, (D, 2 * D)), ("w_zr_h", w_zr_h, (D, 2 * D)),
            ("w_h_x", w_h_x, (D, D)), ("w_hh", w_hh, (D, D)),
            ("w_ne", w_ne, (D, D)), ("w_q", w_q, (D, HKD)),
            ("w_k", w_k, (D, HKD)), ("w_v", w_v, (D, HKD)),
            ("w_o", w_o, (HKD, D)), ("w_ctx", w_ctx, (D, D)),
        ]:
            # weights stored as 2 partition-tiles [128, out]
            t0 = kload(ap_d[0:128, :], [128, shape[1]], BF, nm + "_0")
            t1 = kload(ap_d[128:256, :], [128, shape[1]], BF, nm + "_1")
            W[nm] = [t0, t1]
        W["w_gate"] = [kload(w_gate[k * 128:(k + 1) * 128, :], [128, D], BF,
                             f"w_gate_{k}") for k in range(4)]
        W["neg_gw"] = kload(neg_gw, [1, D], BF, "neg_gw")
        W["g_out_row"] = kload(g_out_row, [1, D], BF, "g_out_row")
        W["neg_beta_out_row"] = kload(neg_beta_out_row, [1, D], BF, "neg_bo_row")
        vecs_sb = [kload(vecs_d[k * 128:(k + 1) * 128, :], [128, NVEC], FP32,
                         f"vecs_{k}") for k in range(2)]
        bzr_sb = [kload(b_zr_d[k * 128:(k + 1) * 128, :], [128, 1], FP32,
                        f"bzr_{k}") for k in range(4)]
        smask_sb = [[kload(smask_d[m, t], [128, H * M], BF, f"smask_{m}_{t}")
                     for t in range(2)] for m in range(M)]
        amask_sb = [[kload(amask_d[m, t], [H * M, 128], BF, f"amask_{m}_{t}")
                     for t in range(2)] for m in range(M)]
        summask_sb = kload(summask_d, [H * M, H], BF, "summask")
        replmask_sb = kload(replmask_d, [H, H * M], BF, "replmask")
        ones_128_1 = kload(ones_128_1_d, [128, 1], BF, "ones_128_1")
        ones_row = kload(ones_row_d, [1, GC], BF, "ones_row")
        ones_1_128 = kload(ones_1_128_d, [1, 128], BF, "ones_1_128")
        ident = kload(ident_d, [128, 128], FP32, "ident")

        def vbias(col, kt):
            return vecs_sb[kt][:, VEC[col]:VEC[col] + 1]

        # ---------------- per-group pipeline ----------------
        for g in range(NG):
            # ---- phase T: load naturals, shift-write, transpose ----
            x_nat, h_nat = [], []
            for r4 in range(NR4):
                r = NR4 * g + r4
                xn = nat.tile([128, D], FP32, tag=f"xnat{r4}", bufs=2, name=f"xnat{r4}")
                nc.sync.dma_start(xn[:, :], x_d[r * 128:(r + 1) * 128, :])
                x_nat.append(xn)
                hn = nat.tile([128, D], FP32, tag=f"hnat{r4}", bufs=2, name=f"hnat{r4}")
                nc.sync.dma_start(hn[:, :], h_d[r * 128:(r + 1) * 128, :])
                h_nat.append(hn)
            mem_nat = []
            for r4 in range(NR4):
                r = NR4 * g + r4
                mn = nat.tile([128, M * D], FP32, tag=f"memnat{r4}", name=f"memnat{r4}")
                nc.sync.dma_start(mn[:, :], mem_d[r * 128:(r + 1) * 128, :])
                # memory shift: f32 copy of slots 1..9 -> 0..8, issued on the
                # scalar HWDGE queue so its load-wait doesn't stall sync's FIFO
                nc.scalar.dma_start(nm_d[r * 128:(r + 1) * 128, 0:(M - 1) * D],
                                    mn[:, D:M * D])
                mem_nat.append(mn)

            def transpose_in(srcs, col_off, dst, dst_dt_tag, evac_engine):
                """PE-transpose 4 [128,128] blocks (one per row-tile) of the
                given column range into one [128,512] psum tile, then evac."""
                ps = ps_tile("ps_tr")
                for r4 in range(NR4):
                    nc.tensor.transpose(ps[:, r4 * 128:(r4 + 1) * 128],
                                        srcs[r4][:, col_off:col_off + 128],
                                        ident[:, :])
                if evac_engine is nc.scalar:
                    nc.scalar.copy(dst[:, :], ps[:, :])
                else:
                    evac_engine.tensor_copy(dst[:, :], ps[:, :])

            memT = []  # [m][t] bf16 [128, 512]
            for m in range(M):
                row = []
                for t in range(2):
                    mt = big.tile([128, GC], BF, tag=f"memT_{m}_{t}",
                                  name=f"memT_{m}_{t}")
                    transpose_in(mem_nat, m * D + t * 128, mt, BF, nc.scalar)
                    row.append(mt)
                memT.append(row)
            xT, hT, hTf = [], [], []
            for t in range(2):
                xt = act.tile([128, GC], BF, tag=f"xT{t}", bufs=2, name=f"xT{t}")
                transpose_in(x_nat, t * 128, xt, BF, nc.scalar)
                xT.append(xt)
            for t in range(2):
                ps = ps_tile("ps_trh")
                for r4 in range(NR4):
                    nc.tensor.transpose(ps[:, r4 * 128:(r4 + 1) * 128],
                                        h_nat[r4][:, t * 128:t * 128 + 128],
                                        ident[:, :])
                ht = act.tile([128, GC], BF, tag=f"hT{t}", bufs=2, name=f"hT{t}")
                nc.scalar.copy(ht[:, :], ps[:, :])
                htf = act.tile([128, GC], FP32, tag=f"hTf{t}", name=f"hTf{t}")
                nc.vector.tensor_copy(htf[:, :], ps[:, :])
                hT.append(ht)
                hTf.append(htf)

            def mm_pair(lhs_tiles, out_slice, rhs_tiles, psum_tile, mt,
                        start=True, stop=True):
                """psum_tile += sum_kt lhs_tiles[kt][:, out_slice].T @ rhs_tiles[kt]"""
                nkt = len(lhs_tiles)
                for kt in range(nkt):
                    nc.tensor.matmul(
                        psum_tile[:, :],
                        lhs_tiles[kt][:, out_slice],
                        rhs_tiles[kt][:, :],
                        start=(start and kt == 0),
                        stop=(stop and kt == nkt - 1),
                    )

            # ---- GRU ----
            # z/r gates: psum = x-part + h-part, then sigmoid (f32 out)
            zr = []
            for mt in range(4):
                ps = ps_tile(f"ps_zr{mt}")
                sl = slice(mt * 128, (mt + 1) * 128)
                mm_pair(W["w_zr_x"], sl, xT, ps, mt, start=True, stop=False)
                mm_pair(W["w_zr_h"], sl, hT, ps, mt, start=False, stop=True)
                zr_t = act.tile([128, GC], FP32, tag=f"zr{mt}", name=f"zr{mt}")
                nc.scalar.activation(zr_t[:, :], ps[:, :], ACTF.Sigmoid,
                                     bias=bzr_sb[mt][:, :], scale=1.0)
                zr.append(zr_t)
            z_f, r_f = zr[0:2], zr[2:4]

            # candidate: hc = tanh(xh + r*(hh + b_hh))
            hc_f, gru_f, gru_b16 = [], [], []
            for mt in range(2):
                sl = slice(mt * 128, (mt + 1) * 128)
                ps_xh = ps_tile(f"ps_xh{mt}")
                mm_pair(W["w_h_x"], sl, xT, ps_xh, mt)
                ps_hh = ps_tile(f"ps_hh{mt}")
                mm_pair(W["w_hh"], sl, hT, ps_hh, mt)
                t1 = act.tile([128, GC], FP32, tag="fscr", bufs=3, name=f"t1_{mt}")
                # t1 = (hh + b_hh) * r
                nc.vector.scalar_tensor_tensor(t1[:, :], ps_hh[:, :],
                                               vbias("b_hh", mt), r_f[mt][:, :],
                                               op0=ALU.add, op1=ALU.mult)
                t2 = act.tile([128, GC], FP32, tag="fscr", bufs=3, name=f"t2_{mt}")
                nc.vector.tensor_tensor(t2[:, :], t1[:, :], ps_xh[:, :], ALU.add)
                hc = act.tile([128, GC], FP32, tag=f"hc{mt}", name=f"hc{mt}")
                nc.scalar.activation(hc[:, :], t2[:, :], ACTF.Tanh,
                                     bias=vbias("b_xh", mt), scale=1.0)
                hc_f.append(hc)
                # gru = hc + z*(h_prev - hc)   (f32 carrier)
                d1 = act.tile([128, GC], FP32, tag="fscr", bufs=3, name=f"d1_{mt}")
                nc.gpsimd.tensor_tensor(d1[:, :], hTf[mt][:, :], hc[:, :], ALU.subtract)
                e1 = act.tile([128, GC], FP32, tag="fscr", bufs=3, name=f"e1_{mt}")
                nc.gpsimd.tensor_tensor(e1[:, :], z_f[mt][:, :], d1[:, :], ALU.mult)
                gr = act.tile([128, GC], FP32, tag=f"gru{mt}", name=f"gru{mt}")
                nc.gpsimd.tensor_tensor(gr[:, :], hc[:, :], e1[:, :], ALU.add)
                gru_f.append(gr)
                grb = act.tile([128, GC], BF, tag=f"grub{mt}", name=f"grub{mt}")
                nc.vector.tensor_copy(grb[:, :], gr[:, :])
                gru_b16.append(grb)

            # ---- q (pre-scaled by 1/8) ----
            q_sb = []
            for mt in range(2):
                sl = slice(mt * 128, (mt + 1) * 128)
                ps = ps_tile(f"ps_q{mt}")
                mm_pair(W["w_q"], sl, gru_b16, ps, mt)
                qs = act.tile([128, GC], BF, tag=f"q{mt}", name=f"q{mt}")
                nc.scalar.activation(qs[:, :], ps[:, :], ACTF.Identity,
                                     bias=vbias("bq_s", mt),
                                     scale=float(1.0 / np.sqrt(DK)))
                q_sb.append(qs)

            # ---- attention loop 1: k + scores ----
            ps_sc = pst.tile([H * M, GC], FP32, tag="psc", bufs=1, name="ps_scores")
            for m in range(M):
                for t in range(2):
                    sl = slice(t * 128, (t + 1) * 128)
                    ps_k = ps_tile(f"ps_k{m}{t}")
                    mm_pair(W["w_k"], sl, memT[m], ps_k, t)
                    prod = act.tile([128, GC], BF, tag=f"sprod{t}",
                                    name=f"sprod{m}{t}")
                    nc.vector.tensor_tensor(prod[:, :], q_sb[t][:, :],
                                            ps_k[:, :], ALU.mult)
                    nc.tensor.matmul(ps_sc[:, :], smask_sb[m][t][:, :],
                                     prod[:, :],
                                     start=(m == 0 and t == 0),
                                     stop=(m == M - 1 and t == 1))

            # ---- softmax over m (no max subtraction; |scores| < 0.3) ----
            e_sb = act.tile([H * M, GC], BF, tag="e_sb", name="e_sb")
            nc.scalar.activation(e_sb[:, :], ps_sc[:, :], ACTF.Exp)
            ps_sum = ps_tile("ps_sum", [H, GC])
            nc.tensor.matmul(ps_sum[:, :], summask_sb[:, :], e_sb[:, :],
                             start=True, stop=True)
            rec_f = sml.tile([H, GC], FP32, tag="recf", name="rec_f")
            nc.vector.reciprocal_approx_fast(rec_f[:, :], ps_sum[:, :])
            rec_sb = sml.tile([H, GC], BF, tag="rec", name="rec_sb")
            nc.vector.tensor_copy(rec_sb[:, :], rec_f[:, :])
            ps_rr = ps_tile("ps_rr", [H * M, GC])
            nc.tensor.matmul(ps_rr[:, :], replmask_sb[:, :], rec_sb[:, :],
                             start=True, stop=True)
            attn_sb = act.tile([H * M, GC], BF, tag="attn", name="attn_sb")
            nc.vector.tensor_tensor(attn_sb[:, :], e_sb[:, :], ps_rr[:, :],
                                    ALU.mult)

            # ---- new_entry (T-layout f32) ----
            neT = []
            for mt in range(2):
                sl = slice(mt * 128, (mt + 1) * 128)
                ps = ps_tile(f"ps_ne{mt}")
                mm_pair(W["w_ne"], sl, xT, ps, mt)
                ne = act.tile([128, GC], FP32, tag=f"neT{mt}", name=f"neT{mt}")
                nc.scalar.activation(ne[:, :], ps[:, :], ACTF.Identity,
                                     bias=vbias("b_ne", mt), scale=1.0)
                neT.append(ne)


            # ---- attention loop 2: v, replicate attn, weighted accumulate ----
            # m-outer so memT[m] frees progressively (lets next group's
            # transposes start); slab chunk for m is [t0|t1] at m*1024.
            MC = 2 * GC
            slab = big.tile([128, 5 * MC], BF, tag="pslab", name="pslab")
            for m in range(M):
                tmp = None
                if m >= 5:
                    tmp = big.tile([128, MC], BF, tag="ptmp", bufs=2,
                                   name=f"ptmp{m}")
                for t in range(2):
                    sl = slice(t * 128, (t + 1) * 128)
                    ps_v = ps_tile(f"ps_v{m}{t}")
                    mm_pair(W["w_v"], sl, memT[m], ps_v, t)
                    ps_er = ps_tile(f"ps_er{m}{t}")
                    nc.tensor.matmul(ps_er[:, :], amask_sb[m][t][:, :],
                                     attn_sb[:, :], start=True, stop=True)
                    er_sb = sml.tile([128, GC], BF, tag="er", bufs=2,
                                     name=f"er{m}{t}")
                    nc.scalar.copy(er_sb[:, :], ps_er[:, :])
                    dst = (slab[:, m * MC + t * GC:m * MC + (t + 1) * GC]
                           if m < 5 else tmp[:, t * GC:(t + 1) * GC])
                    nc.vector.tensor_tensor(dst, er_sb[:, :], ps_v[:, :],
                                            ALU.mult)
                if m >= 5:
                    c = (m - 5) * MC
                    eng = nc.gpsimd if (m % 2) else nc.vector
                    eng.tensor_tensor(slab[:, c:c + MC], slab[:, c:c + MC],
                                      tmp[:, :], ALU.add)
            # tree over remaining 5 chunks
            nc.gpsimd.tensor_tensor(slab[:, 0:2 * MC], slab[:, 0:2 * MC],
                                    slab[:, 2 * MC:4 * MC], ALU.add)
            nc.vector.tensor_tensor(slab[:, 0:MC], slab[:, 0:MC],
                                    slab[:, MC:2 * MC], ALU.add)
            nc.gpsimd.tensor_tensor(slab[:, 0:MC], slab[:, 0:MC],
                                    slab[:, 4 * MC:5 * MC], ALU.add)
            U_sb = []
            for t in range(2):
                u_t = act.tile([128, GC], BF, tag=f"U{t}", name=f"U{t}")
                nc.vector.tensor_copy(u_t[:, :], slab[:, t * GC:(t + 1) * GC])
                U_sb.append(u_t)

            # ---- context = U @ Wo + bo'; keep f32 + bf16 copies ----
            ctx_b16, ps_ctx_list = [], []
            for mt in range(2):
                sl = slice(mt * 128, (mt + 1) * 128)
                ps = ps_tile(f"ps_ctx{mt}")
                for kt in range(2):
                    nc.tensor.matmul(ps[:, :], W["w_o"][kt][:, sl],
                                     U_sb[kt][:, 0:GC],
                                     start=(kt == 0), stop=(kt == 1))
                cb = act.tile([128, GC], BF, tag=f"ctxb{mt}", name=f"ctxb{mt}")
                nc.scalar.activation(cb[:, :], ps[:, :], ACTF.Identity,
                                     bias=vbias("bo_p", mt), scale=1.0)
                ctx_b16.append(cb)

            # ---- LN1 stats (over partitions via ones-matmul) ----
            def ln_stats(x_b16_tiles, sq_tag):
                ps_s1 = ps_tile("ps_s1_" + sq_tag, [1, GC])
                for kt in range(2):
                    nc.tensor.matmul(ps_s1[:, :], ones_128_1[:, :],
                                     x_b16_tiles[kt][:, :],
                                     start=(kt == 0), stop=(kt == 1))
                sq = [act.tile([128, GC], BF, tag=f"sq{kt}",
                               name=f"{sq_tag}{kt}") for kt in range(2)]
                for kt in range(2):
                    nc.vector.tensor_tensor(sq[kt][:, :], x_b16_tiles[kt][:, :],
                                            x_b16_tiles[kt][:, :], ALU.mult)
                ps_s2 = ps_tile("ps_s2_" + sq_tag, [1, GC])
                for kt in range(2):
                    nc.tensor.matmul(ps_s2[:, :], ones_128_1[:, :],
                                     sq[kt][:, :], start=(kt == 0), stop=(kt == 1))
                # A = D / sqrt(D*S2 - S1^2 + D^2*eps);  B = S1 / sqrt(...) = mu*A
                s1sq = sml.tile([1, GC], FP32, tag="lnscr", bufs=2, name="s1sq_" + sq_tag)
                nc.scalar.activation(s1sq[:, :], ps_s1[:, :], ACTF.Square)
                var_t = sml.tile([1, GC], FP32, tag="lnscr", bufs=2, name="var_" + sq_tag)
                nc.vector.scalar_tensor_tensor(var_t[:, :], ps_s2[:, :],
                                               float(D), s1sq[:, :],
                                               op0=ALU.mult, op1=ALU.subtract)
                sd = sml.tile([1, GC], FP32, tag="lnscr", bufs=2, name="sd_" + sq_tag)
                nc.scalar.activation(sd[:, :], var_t[:, :], ACTF.Sqrt,
                                     bias=vecs_sb[0][0:1, VEC["eps_dd"]:VEC["eps_dd"] + 1],
                                     scale=1.0)
                rc = sml.tile([1, GC], FP32, tag="rc", name="rc_" + sq_tag)
                nc.vector.reciprocal_approx_fast(rc[:, :], sd[:, :])
                A_b = sml.tile([1, GC], BF, tag="A_b", name="A_" + sq_tag)
                with nc.allow_low_precision("LN scale bf16"):
                    nc.vector.tensor_scalar(A_b[:, :], rc[:, :], float(D), None,
                                            op0=ALU.mult)
                B_b = sml.tile([1, GC], BF, tag="B_b", name="B_" + sq_tag)
                nc.vector.tensor_tensor(B_b[:, :], ps_s1[:, :], rc[:, :], ALU.mult)
                return A_b, B_b, ps_s1

            A1, B1, ps_s1_ln1 = ln_stats(ctx_b16, "sqc")
            # S1 in bf16 for the rank-1 mean-subtract matmul (rhs)
            s1_b16 = sml.tile([1, GC], BF, tag="s1b", name="s1_b16")
            nc.vector.tensor_copy(s1_b16[:, :], ps_s1_ln1[:, :])
            # A replicated across 128 partitions
            ps_A1 = ps_tile("ps_A1rep")
            nc.tensor.matmul(ps_A1[:, :], ones_1_128[:, :], A1[:, :],
                             start=True, stop=True)
            A1rep = act.tile([128, GC], BF, tag="A1rep", name="A1rep")
            nc.scalar.copy(A1rep[:, :], ps_A1[:, :])

            # ---- ctx_p = tanh(A1 * (ctx@Wc' - mu*gw) + bw_ctx) ----
            ctxp_f, ctxp_b16 = [], []
            for mt in range(2):
                sl = slice(mt * 128, (mt + 1) * 128)
                ps = ps_tile(f"ps_cp{mt}")
                for kt in range(2):
                    nc.tensor.matmul(ps[:, :], W["w_ctx"][kt][:, sl],
                                     ctx_b16[kt][:, :],
                                     start=(kt == 0), stop=False)
                # accumulate rank-1 -(gw/D) * S1  (= -mu*gw)
                nc.tensor.matmul(ps[:, :], W["neg_gw"][:, sl], s1_b16[:, :],
                                 start=False, stop=True)
                tmul = act.tile([128, GC], BF, tag=f"cpm{mt}", name=f"cpm{mt}")
                nc.vector.tensor_tensor(tmul[:, :], A1rep[:, :], ps[:, :], ALU.mult)
                cpf = act.tile([128, GC], FP32, tag=f"ctxp{mt}", name=f"ctxp{mt}")
                nc.scalar.activation(cpf[:, :], tmul[:, :], ACTF.Tanh,
                                     bias=vbias("bw_ctx", mt), scale=1.0)
                ctxp_f.append(cpf)
                cpb = act.tile([128, GC], BF, tag=f"ctxpb{mt}", name=f"ctxpb{mt}")
                nc.vector.tensor_copy(cpb[:, :], cpf[:, :])
                ctxp_b16.append(cpb)

            # ---- alpha gate ----
            alpha_f = []
            for mt in range(2):
                sl = slice(mt * 128, (mt + 1) * 128)
                ps = ps_tile(f"ps_al{mt}")
                for kt in range(2):
                    nc.tensor.matmul(ps[:, :], W["w_gate"][kt][:, sl],
                                     gru_b16[kt][:, :],
                                     start=(kt == 0), stop=False)
                for kt in range(2):
                    nc.tensor.matmul(ps[:, :], W["w_gate"][2 + kt][:, sl],
                                     ctxp_b16[kt][:, :],
                                     start=False, stop=(kt == 1))
                al = act.tile([128, GC], FP32, tag=f"alpha{mt}", name=f"alpha{mt}")
                nc.scalar.activation(al[:, :], ps[:, :], ACTF.Sigmoid,
                                     bias=vbias("b_gate", mt), scale=1.0)
                alpha_f.append(al)

            # ---- blend (f32 carrier) ----
            blend_f, blend_b16 = [], []
            for mt in range(2):
                d2 = act.tile([128, GC], FP32, tag="fscr", bufs=3, name=f"d2_{mt}")
                nc.gpsimd.tensor_tensor(d2[:, :], ctxp_f[mt][:, :],
                                        gru_f[mt][:, :], ALU.subtract)
                e2 = act.tile([128, GC], FP32, tag="fscr", bufs=3, name=f"e2_{mt}")
                nc.gpsimd.tensor_tensor(e2[:, :], alpha_f[mt][:, :], d2[:, :],
                                        ALU.mult)
                bl = act.tile([128, GC], FP32, tag=f"blend{mt}", name=f"blend{mt}")
                nc.gpsimd.tensor_tensor(bl[:, :], gru_f[mt][:, :], e2[:, :],
                                        ALU.add)
                blend_f.append(bl)
                bb = act.tile([128, GC], BF, tag=f"blendb{mt}", name=f"blendb{mt}")
                nc.vector.tensor_copy(bb[:, :], bl[:, :])
                blend_b16.append(bb)

            # ---- LN2 -> h_corr (f32), with g_out/beta_out via rank-1 ----
            A2, B2, _ = ln_stats(blend_b16, "sqb")
            hcT = []
            for mt in range(2):
                sl = slice(mt * 128, (mt + 1) * 128)
                ps_Ag = ps_tile(f"ps_Ag{mt}")
                nc.tensor.matmul(ps_Ag[:, :], W["g_out_row"][:, sl], A2[:, :],
                                 start=True, stop=True)
                ps_Bg = ps_tile(f"ps_Bg{mt}")
                nc.tensor.matmul(ps_Bg[:, :], W["g_out_row"][:, sl], B2[:, :],
                                 start=True, stop=False)
                # -beta_out * ones-row: rhs must be [1, GC] of ones; reuse A2's
                # trick: use ones_1_128 row? Need bf16 [1, GC] ones: use
                # replicated constant via memset-free approach: matmul with
                # lhsT = -beta_out row and rhs = ones_row_b16.
                nc.tensor.matmul(ps_Bg[:, :], W["neg_beta_out_row"][:, sl],
                                 ones_row[:, :], start=False, stop=True)
                t3 = act.tile([128, GC], FP32, tag="fscr", bufs=3, name=f"t3_{mt}")
                nc.vector.tensor_tensor(t3[:, :], blend_f[mt][:, :],
                                        ps_Ag[:, :], ALU.mult)
                hct = act.tile([128, GC], FP32, tag=f"hcT{mt}", name=f"hcT{mt}")
                nc.vector.tensor_tensor(hct[:, :], t3[:, :], ps_Bg[:, :],
                                        ALU.subtract)
                hcT.append(hct)

            # ---- transpose outputs back to natural + DMA out ----
            for r4 in range(NR4):
                r = NR4 * g + r4
                ps = ps_tile(f"ps_otr{r4}", [128, D])
                for mt in range(2):
                    nc.tensor.transpose(ps[:, mt * 128:(mt + 1) * 128],
                                        hcT[mt][:, r4 * 128:(r4 + 1) * 128],
                                        ident[:, :])
                hc_nat = sml.tile([128, D], FP32, tag="hcnat", bufs=2, name=f"hcnat{r4}")
                nc.vector.tensor_copy(hc_nat[:, :], ps[:, :])
                nc.scalar.dma_start(hc_d[r * 128:(r + 1) * 128, :], hc_nat[:, :])
                ps2 = ps_tile(f"ps_otr2{r4}", [128, D])
                for mt in range(2):
                    nc.tensor.transpose(ps2[:, mt * 128:(mt + 1) * 128],
                                        neT[mt][:, r4 * 128:(r4 + 1) * 128],
                                        ident[:, :])
                ne_nat = sml.tile([128, D], FP32, tag="nenat", bufs=2, name=f"nenat{r4}")
                nc.vector.tensor_copy(ne_nat[:, :], ps2[:, :])
                nc.scalar.dma_start(nm_d[r * 128:(r + 1) * 128, (M - 1) * D:M * D],
                                      ne_nat[:, :])

    nc.compile()
    return nc


def _get_kernel():
    if "nc" not in _BUILD_CACHE:
        _BUILD_CACHE["nc"] = build_kernel()
    return _BUILD_CACHE["nc"]


def kernel(**inputs):
    nc = _get_kernel()
    w = _prep_weights(inputs)
    x = np.ascontiguousarray(np.asarray(inputs["inputs"], np.float32))
    h = np.ascontiguousarray(np.asarray(inputs["h_prev"], np.float32))
    mem = np.ascontiguousarray(np.asarray(inputs["memory_flat"], np.float32))

    in_maps = []
    for c in range(N_CORES):
        s = slice(c * BS, (c + 1) * BS)
        im = {"x": x[s], "h": h[s], "mem": mem[s]}
        for k in ("w_zr_x", "w_zr_h", "w_h_x", "w_hh", "w_ne", "w_q", "w_k",
                  "w_v", "w_o", "w_ctx", "w_gate", "neg_gw", "g_out_row",
                  "neg_beta_out_row", "vecs", "b_zr", "smask", "amask",
                  "summask", "replmask", "ones_128_1", "ones_1_128",
                  "ones_row", "ident_f32"):
            im[k] = w[k]
        in_maps.append(im)

    res = run_bass_kernel_spmd(nc, in_maps, core_ids=list(range(N_CORES)))
    h_corr = np.concatenate([res.results[c]["hcorr"] for c in range(N_CORES)], axis=0)
    new_mem = np.concatenate([res.results[c]["newmem"] for c in range(N_CORES)], axis=0)
    return h_corr, new_mem


# revision 20
# speedup vs baseline: 1.1216x; 1.1216x over previous
"""Trainium2 Bass kernel for nn_MATAPCell (GRU + single-query MHA over per-row
memory + gated blend + memory shift-write).

Contract: kernel(**inputs) takes FULL unsharded fp32 inputs (see shapes below),
shards batch across 8 NeuronCores (pure data parallel, weights replicated),
runs a Bass/Tile kernel per core, and gathers the full outputs.

Returns (h_corr [B,256] f32, new_memory_flat [B,2560] f32) matching reference.

Dataflow per core (BS=1024 rows, two column-groups of 512):
  - activations live TRANSPOSED in SBUF: [feature(partitions), row(free)]
  - natural-layout rows are DMA'd in and PE-transposed (f32) into bf16 tiles
  - all matmuls bf16 (weights host-cast), fp32 PSUM accumulate
  - attention: k = mem@Wk on PE; scores q.k via DVE mult + PE mask-matmul
    partition reduction; softmax (no max-subtract: |scores| < 0.3);
    apply: v on PE, attn replicated across dk-partitions via PE mask-matmul,
    DVE mult, wide-AP tree accumulation
  - h-path carrier (h_prev -> gru_out -> blend -> LN2 -> h_corr) kept f32
    in elementwise ops for accuracy
  - memory shift is a pure SBUF->DRAM f32 copy of the loaded mem tile
  - layernorms computed in T-layout via ones-matmul partition sums + rank-1
    replication matmuls
"""

import sys
import os
import numpy as np

for _p in ("/opt/trn_rl_repo",):
    if os.path.isdir(_p) and _p not in sys.path:
        sys.path.insert(0, _p)

import ml_dtypes
from contextlib import ExitStack

import concourse.bass as bass
import concourse.tile as tile
from concourse import bacc, mybir
from concourse.bass_utils import run_bass_kernel_spmd

BF16 = ml_dtypes.bfloat16
FP32 = mybir.dt.float32
BF = mybir.dt.bfloat16

B, D, M, H, DK = 8192, 256, 10, 4, 64
N_CORES = 8
BS = B // N_CORES            # 1024 rows per core
NG = BS // 512               # column groups per core (512 cols each)
GC = 512                     # cols per group
NR4 = 4                      # row-tiles (128 rows) per group
HKD = H * DK                 # 256
D3 = 3 * D                   # 768
LN_EPS = 1e-3
ALU = mybir.AluOpType
ACTF = mybir.ActivationFunctionType

_BUILD_CACHE = {}


def _blob_layout():
    """(name, rows, cols) entries for the bf16 and f32 constant blobs."""
    b = []
    for nm, cols in [("w_zr_x", 512), ("w_zr_h", 512), ("w_h_x", 256),
                     ("w_hh", 256), ("w_ne", 256), ("w_q", 256), ("w_k", 256),
                     ("w_v", 256), ("w_o", 256), ("w_ctx", 256)]:
        b.append((nm + "_0", 128, cols))
        b.append((nm + "_1", 128, cols))
    for k in range(4):
        b.append((f"w_gate_{k}", 128, 256))
    b += [("neg_gw", 1, 256), ("g_out_row", 1, 256),
          ("neg_beta_out_row", 1, 256), ("ones_row", 1, 512),
          ("ones_1_128", 1, 128), ("ones_128_1", 128, 1),
          ("summask", 40, 4), ("replmask", 4, 40)]
    for m in range(M):
        for t in range(2):
            b.append((f"smask_{m}_{t}", 128, 40))
            b.append((f"amask_{m}_{t}", 40, 128))
    f = [("vecs_0", 128, NVEC), ("vecs_1", 128, NVEC)]
    for k in range(4):
        f.append((f"bzr_{k}", 128, 1))
    f.append(("ident", 128, 128))
    return b, f


def _blob_offsets():
    b, f = _blob_layout()
    bo, off = {}, 0
    for nm, rows, cols in b:
        bo[nm] = (off, rows, cols)
        off += cols
    bcols = off
    fo, off = {}, 0
    for nm, rows, cols in f:
        fo[nm] = (off, rows, cols)
        off += cols
    return bo, bcols, fo, off


VEC = {"b_xh": 0, "b_hh": 1, "bq_s": 2, "bo_p": 3, "bw_ctx": 4, "b_gate": 5,
       "b_ne": 6, "eps_dd": 7}
NVEC = 8
BOFF, BCOLS, FOFF, FCOLS = _blob_offsets()


def _prep_weights(inp):
    """Host-side weight fusion + bf16 casts + blob packing. All small."""
    f = lambda x: np.asarray(x, np.float32)
    W_in = f(inp["W_in"]); b_in = f(inp["b_in"])
    gru_k = f(inp["gru_k"]); gru_rk = f(inp["gru_rk"]); gru_b = f(inp["gru_b"])
    Wq = f(inp["Wq"]).reshape(D, HKD); bq = f(inp["bq"]).reshape(HKD)
    Wk = f(inp["Wk"]).reshape(D, HKD)
    Wv = f(inp["Wv"]).reshape(D, HKD); bv = f(inp["bv"]).reshape(HKD)
    Wo = f(inp["Wo"]).reshape(HKD, D); bo = f(inp["bo"])
    g_attn = f(inp["g_attn"]); beta_attn = f(inp["beta_attn"])
    g_out = f(inp["g_out"]); beta_out = f(inp["beta_out"])
    W_ctx = f(inp["W_ctx"]); b_ctx = f(inp["b_ctx"])
    W_gate = f(inp["W_gate"]); b_gate = f(inp["b_gate"])
    W_mem = f(inp["W_mem"]); b_mem = f(inp["b_mem"])

    t = {}
    for nm, arr in [("w_zr_x", W_in @ gru_k[:, :2 * D]),
                    ("w_zr_h", gru_rk[:, :2 * D]),
                    ("w_h_x", W_in @ gru_k[:, 2 * D:]),
                    ("w_hh", gru_rk[:, 2 * D:]),
                    ("w_ne", W_in @ W_mem), ("w_q", Wq), ("w_k", Wk),
                    ("w_v", Wv), ("w_o", Wo),
                    ("w_ctx", g_attn[:, None] * W_ctx)]:
        t[nm + "_0"] = arr[0:128]
        t[nm + "_1"] = arr[128:256]
    for k in range(4):
        t[f"w_gate_{k}"] = W_gate[k * 128:(k + 1) * 128]
    gw = g_attn @ W_ctx
    t["neg_gw"] = -(gw / np.float32(D))[None, :]
    t["g_out_row"] = g_out[None, :]
    t["neg_beta_out_row"] = -beta_out[None, :]
    t["ones_row"] = np.ones((1, 512), np.float32)
    t["ones_1_128"] = np.ones((1, 128), np.float32)
    t["ones_128_1"] = np.ones((128, 1), np.float32)
    summask = np.zeros((H * M, H), np.float32)
    for h in range(H):
        summask[h * M:(h + 1) * M, h] = 1.0
    t["summask"] = summask
    repl = np.zeros((H, H * M), np.float32)
    for h in range(H):
        repl[h, h * M:(h + 1) * M] = 1.0
    t["replmask"] = repl
    for m in range(M):
        for tt in range(2):
            sm = np.zeros((128, H * M), np.float32)
            am = np.zeros((H * M, 128), np.float32)
            for p in range(128):
                h = 2 * tt + p // DK
                sm[p, h * M + m] = 1.0
                am[h * M + m, p] = 1.0
            t[f"smask_{m}_{tt}"] = sm
            t[f"amask_{m}_{tt}"] = am

    kb = np.zeros((128, BCOLS), np.float32)
    for nm, (off, rows, cols) in BOFF.items():
        kb[:rows, off:off + cols] = t[nm]
    kb = kb.astype(BF16)

    b_zr = gru_b[0, :2 * D] + gru_b[1, :2 * D] + b_in @ gru_k[:, :2 * D]
    b_xh = gru_b[0, 2 * D:] + b_in @ gru_k[:, 2 * D:]
    b_hh = gru_b[1, 2 * D:]
    bq_s = bq / np.float32(np.sqrt(DK))
    bo_p = bv @ Wo + bo
    bw_ctx = beta_attn @ W_ctx + b_ctx
    b_ne = b_in @ W_mem + b_mem
    eps_col = np.full((D,), np.float32(D) * np.float32(D) * np.float32(LN_EPS),
                      np.float32)
    vecs = np.stack([b_xh, b_hh, bq_s, bo_p, bw_ctx, b_gate, b_ne, eps_col],
                    axis=1)
    tf = {"vecs_0": vecs[0:128], "vecs_1": vecs[128:256],
          "ident": np.eye(128, dtype=np.float32)}
    for k in range(4):
        tf[f"bzr_{k}"] = b_zr[k * 128:(k + 1) * 128, None]
    kf = np.zeros((128, FCOLS), np.float32)
    for nm, (off, rows, cols) in FOFF.items():
        kf[:rows, off:off + cols] = tf[nm]
    return {"kb": np.ascontiguousarray(kb), "kf": np.ascontiguousarray(kf)}


def build_kernel():
    nc = bacc.Bacc("TRN2", target_bir_lowering=False, debug=False,
                   num_devices=N_CORES)

    def din(name, shape, dt=FP32):
        return nc.dram_tensor(name, shape, dt, kind="ExternalInput").ap()

    def dout(name, shape, dt=FP32):
        return nc.dram_tensor(name, shape, dt, kind="ExternalOutput").ap()

    x_d = din("x", (BS, D))
    h_d = din("h", (BS, D))
    mem_d = din("mem", (BS, M * D))
    kb_d = din("kb", (128, BCOLS), BF)
    kf_d = din("kf", (128, FCOLS))
    hc_d = dout("hcorr", (BS, D))
    nm_d = dout("newmem", (BS, M * D))

    with tile.TileContext(nc) as tc, ExitStack() as ctx:
        konst = ctx.enter_context(tc.tile_pool(name="konst", bufs=1))
        nat = ctx.enter_context(tc.tile_pool(name="nat", bufs=1))
        big = ctx.enter_context(tc.tile_pool(name="big", bufs=1))
        act = ctx.enter_context(tc.tile_pool(name="act", bufs=1))
        sml = ctx.enter_context(tc.tile_pool(name="sml", bufs=1))
        pst = ctx.enter_context(tc.tile_pool(name="pst", bufs=1, space="PSUM"))

        PS_BUFS = 7

        def ps_tile(name, shape=None):
            return pst.tile(shape or [128, GC], FP32, tag="ps", bufs=PS_BUFS,
                            name=name)

        # ---- constant blobs: 2 DMAs total ----
        kb_sb = konst.tile([128, BCOLS], BF, tag="kb", name="kb_sb")
        nc.sync.dma_start(kb_sb[:, :], kb_d[:, :])
        kf_sb = konst.tile([128, FCOLS], FP32, tag="kf", name="kf_sb")
        nc.sync.dma_start(kf_sb[:, :], kf_d[:, :])

        def KB(nm):
            off, rows, cols = BOFF[nm]
            return kb_sb[0:rows, off:off + cols]

        def KF(nm):
            off, rows, cols = FOFF[nm]
            return kf_sb[0:rows, off:off + cols]

        W = {nm: [KB(nm + "_0"), KB(nm + "_1")]
             for nm in ("w_zr_x", "w_zr_h", "w_h_x", "w_hh", "w_ne", "w_q",
                        "w_k", "w_v", "w_o", "w_ctx")}
        W["w_gate"] = [KB(f"w_gate_{k}") for k in range(4)]
        smask_sb = [[KB(f"smask_{m}_{t}") for t in range(2)] for m in range(M)]
        amask_sb = [[KB(f"amask_{m}_{t}") for t in range(2)] for m in range(M)]
        summask_sb = KB("summask")
        replmask_sb = KB("replmask")
        ones_128_1 = KB("ones_128_1")
        ones_1_128 = KB("ones_1_128")
        ones_row = KB("ones_row")
        vecs_sb = [KF("vecs_0"), KF("vecs_1")]
        bzr_sb = [KF(f"bzr_{k}") for k in range(4)]
        ident = KF("ident")
        W["neg_gw"] = KB("neg_gw")
        W["g_out_row"] = KB("g_out_row")
        W["neg_beta_out_row"] = KB("neg_beta_out_row")

        def vbias(col, kt):
            return vecs_sb[kt][:, VEC[col]:VEC[col] + 1]

        # ---------------- per-group pipeline ----------------
        for g in range(NG):
            # ---- phase T: merged loads, shift store, transposes ----
            rows = slice(g * GC, (g + 1) * GC)
            xn = nat.tile([128, NR4 * D], FP32, tag="xnat", bufs=2, name="xn")
            nc.sync.dma_start(
                xn[:, :], x_d[rows, :].rearrange("(a p) c -> p a c", p=128))
            hn = nat.tile([128, NR4 * D], FP32, tag="hnat", bufs=2, name="hn")
            nc.sync.dma_start(
                hn[:, :], h_d[rows, :].rearrange("(a p) c -> p a c", p=128))
            mn = nat.tile([128, NR4 * M * D], FP32, tag="memnat", name="mn")
            for r4 in range(NR4):
                r = NR4 * g + r4
                nc.sync.dma_start(mn[:, r4 * M * D:(r4 + 1) * M * D],
                                  mem_d[r * 128:(r + 1) * 128, :])
            # memory shift store (scalar HWDGE queue; won't stall sync's FIFO)
            nc.scalar.dma_start(
                nm_d[rows, 0:(M - 1) * D].rearrange("(a p) c -> p a c", p=128),
                mn.rearrange("p (a c) -> p a c", a=NR4)[:, :, D:M * D])

            def transpose_in(src_tile, stride, col_off, dst, evac):
                ps = ps_tile("ps_tr")
                for r4 in range(NR4):
                    o = r4 * stride + col_off
                    nc.tensor.transpose(ps[:, r4 * 128:(r4 + 1) * 128],
                                        src_tile[:, o:o + 128], ident)
                if evac == "scalar":
                    nc.scalar.copy(dst[:, :], ps[:, :])
                else:
                    nc.vector.tensor_copy(dst[:, :], ps[:, :])

            memT = []
            for m in range(M):
                row = []
                for t in range(2):
                    mt = big.tile([128, GC], BF, tag=f"memT_{m}_{t}",
                                  name=f"memT_{m}_{t}")
                    transpose_in(mn, M * D, m * D + t * 128, mt, "scalar")
                    row.append(mt)
                memT.append(row)
            xT = []
            for t in range(2):
                xt = act.tile([128, GC], BF, tag=f"xT{t}", bufs=2, name=f"xT{t}")
                transpose_in(xn, D, t * 128, xt, "scalar")
                xT.append(xt)
            hT, hTf = [], []
            for t in range(2):
                ps = ps_tile("ps_trh")
                for r4 in range(NR4):
                    o = r4 * D + t * 128
                    nc.tensor.transpose(ps[:, r4 * 128:(r4 + 1) * 128],
                                        hn[:, o:o + 128], ident)
                ht = act.tile([128, GC], BF, tag=f"hT{t}", bufs=2, name=f"hT{t}")
                nc.scalar.copy(ht[:, :], ps[:, :])
                htf = act.tile([128, GC], FP32, tag=f"hTf{t}", name=f"hTf{t}")
                nc.vector.tensor_copy(htf[:, :], ps[:, :])
                hT.append(ht)
                hTf.append(htf)

            def mm_pair(lhs_tiles, out_slice, rhs_tiles, psum_tile, mt,
                        start=True, stop=True):
                """psum_tile += sum_kt lhs_tiles[kt][:, out_slice].T @ rhs_tiles[kt]"""
                nkt = len(lhs_tiles)
                for kt in range(nkt):
                    nc.tensor.matmul(
                        psum_tile[:, :],
                        lhs_tiles[kt][:, out_slice],
                        rhs_tiles[kt][:, :],
                        start=(start and kt == 0),
                        stop=(stop and kt == nkt - 1),
                    )

            # ---- GRU ----
            # z/r gates: psum = x-part + h-part, then sigmoid (f32 out)
            zr = []
            for mt in range(4):
                ps = ps_tile(f"ps_zr{mt}")
                sl = slice(mt * 128, (mt + 1) * 128)
                mm_pair(W["w_zr_x"], sl, xT, ps, mt, start=True, stop=False)
                mm_pair(W["w_zr_h"], sl, hT, ps, mt, start=False, stop=True)
                zr_t = act.tile([128, GC], FP32, tag=f"zr{mt}", name=f"zr{mt}")
                nc.scalar.activation(zr_t[:, :], ps[:, :], ACTF.Sigmoid,
                                     bias=bzr_sb[mt][:, :], scale=1.0)
                zr.append(zr_t)
            z_f, r_f = zr[0:2], zr[2:4]

            # candidate: hc = tanh(xh + r*(hh + b_hh))
            hc_f, gru_f, gru_b16 = [], [], []
            for mt in range(2):
                sl = slice(mt * 128, (mt + 1) * 128)
                ps_xh = ps_tile(f"ps_xh{mt}")
                mm_pair(W["w_h_x"], sl, xT, ps_xh, mt)
                ps_hh = ps_tile(f"ps_hh{mt}")
                mm_pair(W["w_hh"], sl, hT, ps_hh, mt)
                t1 = act.tile([128, GC], FP32, tag="fscr", bufs=3, name=f"t1_{mt}")
                # t1 = (hh + b_hh) * r
                nc.vector.scalar_tensor_tensor(t1[:, :], ps_hh[:, :],
                                               vbias("b_hh", mt), r_f[mt][:, :],
                                               op0=ALU.add, op1=ALU.mult)
                t2 = act.tile([128, GC], FP32, tag="fscr", bufs=3, name=f"t2_{mt}")
                nc.vector.tensor_tensor(t2[:, :], t1[:, :], ps_xh[:, :], ALU.add)
                hc = act.tile([128, GC], FP32, tag=f"hc{mt}", name=f"hc{mt}")
                nc.scalar.activation(hc[:, :], t2[:, :], ACTF.Tanh,
                                     bias=vbias("b_xh", mt), scale=1.0)
                hc_f.append(hc)
                # gru = hc + z*(h_prev - hc)   (f32 carrier)
                d1 = act.tile([128, GC], FP32, tag="fscr", bufs=3, name=f"d1_{mt}")
                nc.gpsimd.tensor_tensor(d1[:, :], hTf[mt][:, :], hc[:, :], ALU.subtract)
                e1 = act.tile([128, GC], FP32, tag="fscr", bufs=3, name=f"e1_{mt}")
                nc.gpsimd.tensor_tensor(e1[:, :], z_f[mt][:, :], d1[:, :], ALU.mult)
                gr = act.tile([128, GC], FP32, tag=f"gru{mt}", name=f"gru{mt}")
                nc.gpsimd.tensor_tensor(gr[:, :], hc[:, :], e1[:, :], ALU.add)
                gru_f.append(gr)
                grb = act.tile([128, GC], BF, tag=f"grub{mt}", name=f"grub{mt}")
                nc.vector.tensor_copy(grb[:, :], gr[:, :])
                gru_b16.append(grb)

            # ---- q (pre-scaled by 1/8) ----
            q_sb = []
            for mt in range(2):
                sl = slice(mt * 128, (mt + 1) * 128)
                ps = ps_tile(f"ps_q{mt}")
                mm_pair(W["w_q"], sl, gru_b16, ps, mt)
                qs = act.tile([128, GC], BF, tag=f"q{mt}", name=f"q{mt}")
                nc.scalar.activation(qs[:, :], ps[:, :], ACTF.Identity,
                                     bias=vbias("bq_s", mt),
                                     scale=float(1.0 / np.sqrt(DK)))
                q_sb.append(qs)

            # ---- attention loop 1: k + scores ----
            ps_sc = pst.tile([H * M, GC], FP32, tag="psc", bufs=1, name="ps_scores")
            for m in range(M):
                for t in range(2):
                    sl = slice(t * 128, (t + 1) * 128)
                    ps_k = ps_tile(f"ps_k{m}{t}")
                    mm_pair(W["w_k"], sl, memT[m], ps_k, t)
                    prod = act.tile([128, GC], BF, tag=f"sprod{t}",
                                    name=f"sprod{m}{t}")
                    nc.vector.tensor_tensor(prod[:, :], q_sb[t][:, :],
                                            ps_k[:, :], ALU.mult)
                    nc.tensor.matmul(ps_sc[:, :], smask_sb[m][t][:, :],
                                     prod[:, :],
                                     start=(m == 0 and t == 0),
                                     stop=(m == M - 1 and t == 1))

            # ---- softmax over m (no max subtraction; |scores| < 0.3) ----
            e_sb = act.tile([H * M, GC], BF, tag="e_sb", name="e_sb")
            nc.scalar.activation(e_sb[:, :], ps_sc[:, :], ACTF.Exp)
            ps_sum = ps_tile("ps_sum", [H, GC])
            nc.tensor.matmul(ps_sum[:, :], summask_sb[:, :], e_sb[:, :],
                             start=True, stop=True)
            rec_f = sml.tile([H, GC], FP32, tag="lnscr", bufs=2, name="rec_f")
            nc.vector.reciprocal_approx_fast(rec_f[:, :], ps_sum[:, :])
            rec_sb = sml.tile([H, GC], BF, tag="rec", name="rec_sb")
            nc.vector.tensor_copy(rec_sb[:, :], rec_f[:, :])
            ps_rr = ps_tile("ps_rr", [H * M, GC])
            nc.tensor.matmul(ps_rr[:, :], replmask_sb[:, :], rec_sb[:, :],
                             start=True, stop=True)
            attn_sb = act.tile([H * M, GC], BF, tag="attn", name="attn_sb")
            nc.vector.tensor_tensor(attn_sb[:, :], e_sb[:, :], ps_rr[:, :],
                                    ALU.mult)

            # ---- new_entry (T-layout f32) ----
            neT = []
            for mt in range(2):
                sl = slice(mt * 128, (mt + 1) * 128)
                ps = ps_tile(f"ps_ne{mt}")
                mm_pair(W["w_ne"], sl, xT, ps, mt)
                ne = act.tile([128, GC], FP32, tag=f"neT{mt}", name=f"neT{mt}")
                nc.scalar.activation(ne[:, :], ps[:, :], ACTF.Identity,
                                     bias=vbias("b_ne", mt), scale=1.0)
                neT.append(ne)


            # ---- attention loop 2: v, replicate attn, weighted accumulate ----
            # m-outer so memT[m] frees progressively (lets next group's
            # transposes start); slab chunk for m is [t0|t1] at m*1024.
            MC = 2 * GC
            slab = big.tile([128, 5 * MC], BF, tag="pslab", name="pslab")
            for m in range(M):
                tmp = None
                if m >= 5:
                    tmp = big.tile([128, MC], BF, tag="ptmp", bufs=2,
                                   name=f"ptmp{m}")
                for t in range(2):
                    sl = slice(t * 128, (t + 1) * 128)
                    ps_v = ps_tile(f"ps_v{m}{t}")
                    mm_pair(W["w_v"], sl, memT[m], ps_v, t)
                    ps_er = ps_tile(f"ps_er{m}{t}")
                    nc.tensor.matmul(ps_er[:, :], amask_sb[m][t][:, :],
                                     attn_sb[:, :], start=True, stop=True)
                    er_sb = sml.tile([128, GC], BF, tag="er", bufs=1,
                                     name=f"er{m}{t}")
                    nc.scalar.copy(er_sb[:, :], ps_er[:, :])
                    dst = (slab[:, m * MC + t * GC:m * MC + (t + 1) * GC]
                           if m < 5 else tmp[:, t * GC:(t + 1) * GC])
                    nc.vector.tensor_tensor(dst, er_sb[:, :], ps_v[:, :],
                                            ALU.mult)
                if m >= 5:
                    c = (m - 5) * MC
                    eng = nc.gpsimd if (m % 2) else nc.vector
                    eng.tensor_tensor(slab[:, c:c + MC], slab[:, c:c + MC],
                                      tmp[:, :], ALU.add)
            # tree over remaining 5 chunks
            nc.gpsimd.tensor_tensor(slab[:, 0:2 * MC], slab[:, 0:2 * MC],
                                    slab[:, 2 * MC:4 * MC], ALU.add)
            nc.vector.tensor_tensor(slab[:, 0:MC], slab[:, 0:MC],
                                    slab[:, MC:2 * MC], ALU.add)
            nc.gpsimd.tensor_tensor(slab[:, 0:MC], slab[:, 0:MC],
                                    slab[:, 4 * MC:5 * MC], ALU.add)
            U_sb = []
            for t in range(2):
                u_t = act.tile([128, GC], BF, tag=f"U{t}", name=f"U{t}")
                nc.vector.tensor_copy(u_t[:, :], slab[:, t * GC:(t + 1) * GC])
                U_sb.append(u_t)

            # ---- context = U @ Wo + bo'; keep f32 + bf16 copies ----
            ctx_b16, ps_ctx_list = [], []
            for mt in range(2):
                sl = slice(mt * 128, (mt + 1) * 128)
                ps = ps_tile(f"ps_ctx{mt}")
                for kt in range(2):
                    nc.tensor.matmul(ps[:, :], W["w_o"][kt][:, sl],
                                     U_sb[kt][:, 0:GC],
                                     start=(kt == 0), stop=(kt == 1))
                cb = act.tile([128, GC], BF, tag=f"ctxb{mt}", name=f"ctxb{mt}")
                nc.scalar.activation(cb[:, :], ps[:, :], ACTF.Identity,
                                     bias=vbias("bo_p", mt), scale=1.0)
                ctx_b16.append(cb)

            # ---- LN1 stats (over partitions via ones-matmul) ----
            def ln_stats(x_b16_tiles, sq_tag):
                ps_s1 = ps_tile("ps_s1_" + sq_tag, [1, GC])
                for kt in range(2):
                    nc.tensor.matmul(ps_s1[:, :], ones_128_1[:, :],
                                     x_b16_tiles[kt][:, :],
                                     start=(kt == 0), stop=(kt == 1))
                sq = [act.tile([128, GC], BF, tag=f"sq{kt}",
                               name=f"{sq_tag}{kt}") for kt in range(2)]
                for kt in range(2):
                    nc.vector.tensor_tensor(sq[kt][:, :], x_b16_tiles[kt][:, :],
                                            x_b16_tiles[kt][:, :], ALU.mult)
                ps_s2 = ps_tile("ps_s2_" + sq_tag, [1, GC])
                for kt in range(2):
                    nc.tensor.matmul(ps_s2[:, :], ones_128_1[:, :],
                                     sq[kt][:, :], start=(kt == 0), stop=(kt == 1))
                # A = D / sqrt(D*S2 - S1^2 + D^2*eps);  B = S1 / sqrt(...) = mu*A
                s1sq = sml.tile([1, GC], FP32, tag="lnscr", bufs=2, name="s1sq_" + sq_tag)
                nc.scalar.activation(s1sq[:, :], ps_s1[:, :], ACTF.Square)
                var_t = sml.tile([1, GC], FP32, tag="lnscr", bufs=2, name="var_" + sq_tag)
                nc.vector.scalar_tensor_tensor(var_t[:, :], ps_s2[:, :],
                                               float(D), s1sq[:, :],
                                               op0=ALU.mult, op1=ALU.subtract)
                sd = sml.tile([1, GC], FP32, tag="lnscr", bufs=2, name="sd_" + sq_tag)
                nc.scalar.activation(sd[:, :], var_t[:, :], ACTF.Sqrt,
                                     bias=vecs_sb[0][0:1, VEC["eps_dd"]:VEC["eps_dd"] + 1],
                                     scale=1.0)
                rc = sml.tile([1, GC], FP32, tag="lnscr", bufs=2, name="rc_" + sq_tag)
                nc.vector.reciprocal_approx_fast(rc[:, :], sd[:, :])
                A_b = sml.tile([1, GC], BF, tag="A_b", name="A_" + sq_tag)
                with nc.allow_low_precision("LN scale bf16"):
                    nc.vector.tensor_scalar(A_b[:, :], rc[:, :], float(D), None,
                                            op0=ALU.mult)
                B_b = sml.tile([1, GC], BF, tag="B_b", name="B_" + sq_tag)
                nc.vector.tensor_tensor(B_b[:, :], ps_s1[:, :], rc[:, :], ALU.mult)
                return A_b, B_b, ps_s1

            A1, B1, ps_s1_ln1 = ln_stats(ctx_b16, "sqc")
            # S1 in bf16 for the rank-1 mean-subtract matmul (rhs)
            s1_b16 = sml.tile([1, GC], BF, tag="s1b", name="s1_b16")
            nc.vector.tensor_copy(s1_b16[:, :], ps_s1_ln1[:, :])
            # A replicated across 128 partitions
            ps_A1 = ps_tile("ps_A1rep")
            nc.tensor.matmul(ps_A1[:, :], ones_1_128[:, :], A1[:, :],
                             start=True, stop=True)
            A1rep = act.tile([128, GC], BF, tag="A1rep", name="A1rep")
            nc.scalar.copy(A1rep[:, :], ps_A1[:, :])

            # ---- ctx_p = tanh(A1 * (ctx@Wc' - mu*gw) + bw_ctx) ----
            ctxp_f, ctxp_b16 = [], []
            for mt in range(2):
                sl = slice(mt * 128, (mt + 1) * 128)
                ps = ps_tile(f"ps_cp{mt}")
                for kt in range(2):
                    nc.tensor.matmul(ps[:, :], W["w_ctx"][kt][:, sl],
                                     ctx_b16[kt][:, :],
                                     start=(kt == 0), stop=False)
                # accumulate rank-1 -(gw/D) * S1  (= -mu*gw)
                nc.tensor.matmul(ps[:, :], W["neg_gw"][:, sl], s1_b16[:, :],
                                 start=False, stop=True)
                tmul = act.tile([128, GC], BF, tag=f"cpm{mt}", name=f"cpm{mt}")
                nc.vector.tensor_tensor(tmul[:, :], A1rep[:, :], ps[:, :], ALU.mult)
                cpf = act.tile([128, GC], FP32, tag=f"ctxp{mt}", name=f"ctxp{mt}")
                nc.scalar.activation(cpf[:, :], tmul[:, :], ACTF.Tanh,
                                     bias=vbias("bw_ctx", mt), scale=1.0)
                ctxp_f.append(cpf)
                cpb = act.tile([128, GC], BF, tag=f"ctxpb{mt}", name=f"ctxpb{mt}")
                nc.vector.tensor_copy(cpb[:, :], cpf[:, :])
                ctxp_b16.append(cpb)

            # ---- alpha gate ----
            alpha_f = []
            for mt in range(2):
                sl = slice(mt * 128, (mt + 1) * 128)
                ps = ps_tile(f"ps_al{mt}")
                for kt in range(2):
                    nc.tensor.matmul(ps[:, :], W["w_gate"][kt][:, sl],
                                     gru_b16[kt][:, :],
                                     start=(kt == 0), stop=False)
                for kt in range(2):
                    nc.tensor.matmul(ps[:, :], W["w_gate"][2 + kt][:, sl],
                                     ctxp_b16[kt][:, :],
                                     start=False, stop=(kt == 1))
                al = act.tile([128, GC], FP32, tag=f"alpha{mt}", name=f"alpha{mt}")
                nc.scalar.activation(al[:, :], ps[:, :], ACTF.Sigmoid,
                                     bias=vbias("b_gate", mt), scale=1.0)
                alpha_f.append(al)

            # ---- blend (f32 carrier) ----
            blend_f, blend_b16 = [], []
            for mt in range(2):
                d2 = act.tile([128, GC], FP32, tag="fscr", bufs=3, name=f"d2_{mt}")
                nc.gpsimd.tensor_tensor(d2[:, :], ctxp_f[mt][:, :],
                                        gru_f[mt][:, :], ALU.subtract)
                e2 = act.tile([128, GC], FP32, tag="fscr", bufs=3, name=f"e2_{mt}")
                nc.gpsimd.tensor_tensor(e2[:, :], alpha_f[mt][:, :], d2[:, :],
                                        ALU.mult)
                bl = act.tile([128, GC], FP32, tag=f"blend{mt}", name=f"blend{mt}")
                nc.gpsimd.tensor_tensor(bl[:, :], gru_f[mt][:, :], e2[:, :],
                                        ALU.add)
                blend_f.append(bl)
                bb = act.tile([128, GC], BF, tag=f"blendb{mt}", name=f"blendb{mt}")
                nc.vector.tensor_copy(bb[:, :], bl[:, :])
                blend_b16.append(bb)

            # ---- LN2 -> h_corr (f32), with g_out/beta_out via rank-1 ----
            A2, B2, _ = ln_stats(blend_b16, "sqb")
            hcT = []
            for mt in range(2):
                sl = slice(mt * 128, (mt + 1) * 128)
                ps_Ag = ps_tile(f"ps_Ag{mt}")
                nc.tensor.matmul(ps_Ag[:, :], W["g_out_row"][:, sl], A2[:, :],
                                 start=True, stop=True)
                ps_Bg = ps_tile(f"ps_Bg{mt}")
                nc.tensor.matmul(ps_Bg[:, :], W["g_out_row"][:, sl], B2[:, :],
                                 start=True, stop=False)
                # -beta_out * ones-row: rhs must be [1, GC] of ones; reuse A2's
                # trick: use ones_1_128 row? Need bf16 [1, GC] ones: use
                # replicated constant via memset-free approach: matmul with
                # lhsT = -beta_out row and rhs = ones_row_b16.
                nc.tensor.matmul(ps_Bg[:, :], W["neg_beta_out_row"][:, sl],
                                 ones_row[:, :], start=False, stop=True)
                t3 = act.tile([128, GC], FP32, tag="fscr", bufs=3, name=f"t3_{mt}")
                nc.vector.tensor_tensor(t3[:, :], blend_f[mt][:, :],
                                        ps_Ag[:, :], ALU.mult)
                hct = act.tile([128, GC], FP32, tag=f"hcT{mt}", name=f"hcT{mt}")
                nc.vector.tensor_tensor(hct[:, :], t3[:, :], ps_Bg[:, :],
                                        ALU.subtract)
                hcT.append(hct)

            # ---- transpose outputs back to natural + merged DMA out ----
            hc_nat = sml.tile([128, NR4 * D], FP32, tag="hcnat", bufs=1,
                              name="hc_nat")
            ne_nat = sml.tile([128, NR4 * D], FP32, tag="nenat", bufs=1,
                              name="ne_nat")
            for r4 in range(NR4):
                ps = ps_tile(f"ps_otr{r4}", [128, D])
                for mt in range(2):
                    nc.tensor.transpose(ps[:, mt * 128:(mt + 1) * 128],
                                        hcT[mt][:, r4 * 128:(r4 + 1) * 128],
                                        ident)
                nc.vector.tensor_copy(hc_nat[:, r4 * D:(r4 + 1) * D], ps[:, :])
                ps2 = ps_tile(f"ps_otr2{r4}", [128, D])
                for mt in range(2):
                    nc.tensor.transpose(ps2[:, mt * 128:(mt + 1) * 128],
                                        neT[mt][:, r4 * 128:(r4 + 1) * 128],
                                        ident)
                nc.vector.tensor_copy(ne_nat[:, r4 * D:(r4 + 1) * D],
                                      ps2[:, :])
            nc.scalar.dma_start(
                hc_d[rows, :].rearrange("(a p) c -> p a c", p=128),
                hc_nat[:, :])
            nc.scalar.dma_start(
                nm_d[rows, (M - 1) * D:M * D].rearrange("(a p) c -> p a c", p=128),
                ne_nat[:, :])

    nc.compile()
    return nc


def _get_kernel():
    if "nc" not in _BUILD_CACHE:
        _BUILD_CACHE["nc"] = build_kernel()
    return _BUILD_CACHE["nc"]


def make_in_maps(inputs):
    w = _prep_weights(inputs)
    x = np.ascontiguousarray(np.asarray(inputs["inputs"], np.float32))
    h = np.ascontiguousarray(np.asarray(inputs["h_prev"], np.float32))
    mem = np.ascontiguousarray(np.asarray(inputs["memory_flat"], np.float32))
    in_maps = []
    for c in range(N_CORES):
        s = slice(c * BS, (c + 1) * BS)
        in_maps.append({"x": x[s], "h": h[s], "mem": mem[s],
                        "kb": w["kb"], "kf": w["kf"]})
    return in_maps


def kernel(**inputs):
    nc = _get_kernel()
    in_maps = make_in_maps(inputs)
    res = run_bass_kernel_spmd(nc, in_maps, core_ids=list(range(N_CORES)))
    h_corr = np.concatenate([res.results[c]["hcorr"] for c in range(N_CORES)],
                            axis=0)
    new_mem = np.concatenate([res.results[c]["newmem"] for c in range(N_CORES)],
                             axis=0)
    return h_corr, new_mem


# revision 22
# speedup vs baseline: 1.1567x; 1.0312x over previous
"""Trainium2 Bass kernel for nn_MATAPCell (GRU + single-query MHA over per-row
memory + gated blend + memory shift-write).

Contract: kernel(**inputs) takes FULL unsharded fp32 inputs (see shapes below),
shards batch across 8 NeuronCores (pure data parallel, weights replicated),
runs a Bass/Tile kernel per core, and gathers the full outputs.

Returns (h_corr [B,256] f32, new_memory_flat [B,2560] f32) matching reference.

Dataflow per core (BS=1024 rows, two column-groups of 512):
  - activations live TRANSPOSED in SBUF: [feature(partitions), row(free)]
  - natural-layout rows are DMA'd in and PE-transposed (f32) into bf16 tiles
  - all matmuls bf16 (weights host-cast), fp32 PSUM accumulate
  - attention: k = mem@Wk on PE; scores q.k via DVE mult + PE mask-matmul
    partition reduction; softmax (no max-subtract: |scores| < 0.3);
    apply: v on PE, attn replicated across dk-partitions via PE mask-matmul,
    DVE mult, wide-AP tree accumulation
  - h-path carrier (h_prev -> gru_out -> blend -> LN2 -> h_corr) kept f32
    in elementwise ops for accuracy
  - memory shift is a pure SBUF->DRAM f32 copy of the loaded mem tile
  - layernorms computed in T-layout via ones-matmul partition sums + rank-1
    replication matmuls
"""

import sys
import os
import numpy as np

for _p in ("/opt/trn_rl_repo",):
    if os.path.isdir(_p) and _p not in sys.path:
        sys.path.insert(0, _p)

import ml_dtypes
from contextlib import ExitStack

import concourse.bass as bass
import concourse.tile as tile
from concourse import bacc, mybir
from concourse.bass_utils import run_bass_kernel_spmd

BF16 = ml_dtypes.bfloat16
FP32 = mybir.dt.float32
BF = mybir.dt.bfloat16

B, D, M, H, DK = 8192, 256, 10, 4, 64
N_CORES = 8
BS = B // N_CORES            # 1024 rows per core
NG = BS // 512               # column groups per core (512 cols each)
GC = 512                     # cols per group
NR4 = 4                      # row-tiles (128 rows) per group
HKD = H * DK                 # 256
D3 = 3 * D                   # 768
LN_EPS = 1e-3
ALU = mybir.AluOpType
ACTF = mybir.ActivationFunctionType

_BUILD_CACHE = {}


def _blob_layout():
    """(name, rows, cols) entries for the bf16 and f32 constant blobs."""
    b = []
    for nm, cols in [("w_zr_x", 512), ("w_zr_h", 512), ("w_h_x", 256),
                     ("w_hh", 256), ("w_ne", 256), ("w_q", 256), ("w_k", 256),
                     ("w_v", 256), ("w_o", 256), ("w_ctx", 256)]:
        b.append((nm + "_0", 128, cols))
        b.append((nm + "_1", 128, cols))
    for k in range(4):
        b.append((f"w_gate_{k}", 128, 256))
    b += [("neg_gw", 1, 256), ("g_out_row", 1, 256),
          ("neg_beta_out_row", 1, 256), ("ones_row", 1, 512),
          ("ones_1_128", 1, 128), ("ones_128_1", 128, 1),
          ("summask", 40, 4), ("replmask", 4, 40)]
    for m in range(M):
        for t in range(2):
            b.append((f"smask_{m}_{t}", 128, 40))
            b.append((f"amask_{m}_{t}", 40, 128))
    f = [("vecs_0", 128, NVEC), ("vecs_1", 128, NVEC)]
    for k in range(4):
        f.append((f"bzr_{k}", 128, 1))
    f.append(("ident", 128, 128))
    return b, f


def _blob_offsets():
    b, f = _blob_layout()
    bo, off = {}, 0
    for nm, rows, cols in b:
        bo[nm] = (off, rows, cols)
        off += cols
    bcols = off
    fo, off = {}, 0
    for nm, rows, cols in f:
        fo[nm] = (off, rows, cols)
        off += cols
    return bo, bcols, fo, off


VEC = {"b_xh": 0, "b_hh": 1, "bq_s": 2, "bo_p": 3, "bw_ctx": 4, "b_gate": 5,
       "b_ne": 6, "eps_dd": 7}
NVEC = 8
BOFF, BCOLS, FOFF, FCOLS = _blob_offsets()


def _prep_weights(inp):
    """Host-side weight fusion + bf16 casts + blob packing. All small."""
    f = lambda x: np.asarray(x, np.float32)
    W_in = f(inp["W_in"]); b_in = f(inp["b_in"])
    gru_k = f(inp["gru_k"]); gru_rk = f(inp["gru_rk"]); gru_b = f(inp["gru_b"])
    Wq = f(inp["Wq"]).reshape(D, HKD); bq = f(inp["bq"]).reshape(HKD)
    Wk = f(inp["Wk"]).reshape(D, HKD)
    Wv = f(inp["Wv"]).reshape(D, HKD); bv = f(inp["bv"]).reshape(HKD)
    Wo = f(inp["Wo"]).reshape(HKD, D); bo = f(inp["bo"])
    g_attn = f(inp["g_attn"]); beta_attn = f(inp["beta_attn"])
    g_out = f(inp["g_out"]); beta_out = f(inp["beta_out"])
    W_ctx = f(inp["W_ctx"]); b_ctx = f(inp["b_ctx"])
    W_gate = f(inp["W_gate"]); b_gate = f(inp["b_gate"])
    W_mem = f(inp["W_mem"]); b_mem = f(inp["b_mem"])

    t = {}
    for nm, arr in [("w_zr_x", W_in @ gru_k[:, :2 * D]),
                    ("w_zr_h", gru_rk[:, :2 * D]),
                    ("w_h_x", W_in @ gru_k[:, 2 * D:]),
                    ("w_hh", gru_rk[:, 2 * D:]),
                    ("w_ne", W_in @ W_mem), ("w_q", Wq), ("w_k", Wk),
                    ("w_v", Wv), ("w_o", Wo),
                    ("w_ctx", g_attn[:, None] * W_ctx)]:
        t[nm + "_0"] = arr[0:128]
        t[nm + "_1"] = arr[128:256]
    for k in range(4):
        t[f"w_gate_{k}"] = W_gate[k * 128:(k + 1) * 128]
    gw = g_attn @ W_ctx
    t["neg_gw"] = -(gw / np.float32(D))[None, :]
    t["g_out_row"] = g_out[None, :]
    t["neg_beta_out_row"] = -beta_out[None, :]
    t["ones_row"] = np.ones((1, 512), np.float32)
    t["ones_1_128"] = np.ones((1, 128), np.float32)
    t["ones_128_1"] = np.ones((128, 1), np.float32)
    summask = np.zeros((H * M, H), np.float32)
    for h in range(H):
        summask[h * M:(h + 1) * M, h] = 1.0
    t["summask"] = summask
    repl = np.zeros((H, H * M), np.float32)
    for h in range(H):
        repl[h, h * M:(h + 1) * M] = 1.0
    t["replmask"] = repl
    for m in range(M):
        for tt in range(2):
            sm = np.zeros((128, H * M), np.float32)
            am = np.zeros((H * M, 128), np.float32)
            for p in range(128):
                h = 2 * tt + p // DK
                sm[p, h * M + m] = 1.0
                am[h * M + m, p] = 1.0
            t[f"smask_{m}_{tt}"] = sm
            t[f"amask_{m}_{tt}"] = am

    kb = np.zeros((128, BCOLS), np.float32)
    for nm, (off, rows, cols) in BOFF.items():
        kb[:rows, off:off + cols] = t[nm]
    kb = kb.astype(BF16)

    b_zr = gru_b[0, :2 * D] + gru_b[1, :2 * D] + b_in @ gru_k[:, :2 * D]
    b_xh = gru_b[0, 2 * D:] + b_in @ gru_k[:, 2 * D:]
    b_hh = gru_b[1, 2 * D:]
    bq_s = bq / np.float32(np.sqrt(DK))
    bo_p = bv @ Wo + bo
    bw_ctx = beta_attn @ W_ctx + b_ctx
    b_ne = b_in @ W_mem + b_mem
    eps_col = np.full((D,), np.float32(D) * np.float32(D) * np.float32(LN_EPS),
                      np.float32)
    vecs = np.stack([b_xh, b_hh, bq_s, bo_p, bw_ctx, b_gate, b_ne, eps_col],
                    axis=1)
    tf = {"vecs_0": vecs[0:128], "vecs_1": vecs[128:256],
          "ident": np.eye(128, dtype=np.float32)}
    for k in range(4):
        tf[f"bzr_{k}"] = b_zr[k * 128:(k + 1) * 128, None]
    kf = np.zeros((128, FCOLS), np.float32)
    for nm, (off, rows, cols) in FOFF.items():
        kf[:rows, off:off + cols] = tf[nm]
    return {"kb": np.ascontiguousarray(kb), "kf": np.ascontiguousarray(kf)}


def build_kernel():
    nc = bacc.Bacc("TRN2", target_bir_lowering=False, debug=False,
                   num_devices=N_CORES)

    def din(name, shape, dt=FP32):
        return nc.dram_tensor(name, shape, dt, kind="ExternalInput").ap()

    def dout(name, shape, dt=FP32):
        return nc.dram_tensor(name, shape, dt, kind="ExternalOutput").ap()

    x_d = din("x", (BS, D))
    h_d = din("h", (BS, D))
    mem_d = din("mem", (BS, M * D))
    kb_d = din("kb", (128, BCOLS), BF)
    kf_d = din("kf", (128, FCOLS))
    hc_d = dout("hcorr", (BS, D))
    nm_d = dout("newmem", (BS, M * D))

    with tile.TileContext(nc) as tc, ExitStack() as ctx:
        konst = ctx.enter_context(tc.tile_pool(name="konst", bufs=1))
        nat = ctx.enter_context(tc.tile_pool(name="nat", bufs=1))
        big = ctx.enter_context(tc.tile_pool(name="big", bufs=1))
        act = ctx.enter_context(tc.tile_pool(name="act", bufs=1))
        sml = ctx.enter_context(tc.tile_pool(name="sml", bufs=1))
        pst = ctx.enter_context(tc.tile_pool(name="pst", bufs=1, space="PSUM"))

        PS_BUFS = 7

        def ps_tile(name, shape=None):
            return pst.tile(shape or [128, GC], FP32, tag="ps", bufs=PS_BUFS,
                            name=name)

        # ---- constant blobs: 2 DMAs total ----
        kb_sb = konst.tile([128, BCOLS], BF, tag="kb", name="kb_sb")
        nc.sync.dma_start(kb_sb[:, :], kb_d[:, :])
        kf_sb = konst.tile([128, FCOLS], FP32, tag="kf", name="kf_sb")
        nc.sync.dma_start(kf_sb[:, :], kf_d[:, :])

        def KB(nm):
            off, rows, cols = BOFF[nm]
            return kb_sb[0:rows, off:off + cols]

        def KF(nm):
            off, rows, cols = FOFF[nm]
            return kf_sb[0:rows, off:off + cols]

        W = {nm: [KB(nm + "_0"), KB(nm + "_1")]
             for nm in ("w_zr_x", "w_zr_h", "w_h_x", "w_hh", "w_ne", "w_q",
                        "w_k", "w_v", "w_o", "w_ctx")}
        W["w_gate"] = [KB(f"w_gate_{k}") for k in range(4)]
        smask_sb = [[KB(f"smask_{m}_{t}") for t in range(2)] for m in range(M)]
        amask_sb = [[KB(f"amask_{m}_{t}") for t in range(2)] for m in range(M)]
        summask_sb = KB("summask")
        replmask_sb = KB("replmask")
        ones_128_1 = KB("ones_128_1")
        ones_1_128 = KB("ones_1_128")
        ones_row = KB("ones_row")
        vecs_sb = [KF("vecs_0"), KF("vecs_1")]
        bzr_sb = [KF(f"bzr_{k}") for k in range(4)]
        ident = KF("ident")
        W["neg_gw"] = KB("neg_gw")
        W["g_out_row"] = KB("g_out_row")
        W["neg_beta_out_row"] = KB("neg_beta_out_row")

        def vbias(col, kt):
            return vecs_sb[kt][:, VEC[col]:VEC[col] + 1]

        # ---------------- per-group pipeline, phase-interleaved ----------------
        ST = [dict() for _ in range(NG)]

        def phase_T(g):
            s = ST[g]
            rows = slice(g * GC, (g + 1) * GC)
            xn = nat.tile([128, NR4 * D], FP32, tag="xnat", bufs=2, name="xn")
            nc.sync.dma_start(
                xn[:, :], x_d[rows, :].rearrange("(a p) c -> p a c", p=128))
            hn = nat.tile([128, NR4 * D], FP32, tag="hnat", bufs=2, name="hn")
            nc.sync.dma_start(
                hn[:, :], h_d[rows, :].rearrange("(a p) c -> p a c", p=128))
            mn = nat.tile([128, NR4 * M * D], FP32, tag="memnat", name="mn")
            for r4 in range(NR4):
                r = NR4 * g + r4
                nc.sync.dma_start(mn[:, r4 * M * D:(r4 + 1) * M * D],
                                  mem_d[r * 128:(r + 1) * 128, :])
            # memory shift store (scalar HWDGE queue; won't stall sync's FIFO)
            nc.scalar.dma_start(
                nm_d[rows, 0:(M - 1) * D].rearrange("(a p) c -> p a c", p=128),
                mn.rearrange("p (a c) -> p a c", a=NR4)[:, :, D:M * D])

            def transpose_in(src_tile, stride, col_off, dst, evac):
                ps = ps_tile("ps_tr")
                for r4 in range(NR4):
                    o = r4 * stride + col_off
                    nc.tensor.transpose(ps[:, r4 * 128:(r4 + 1) * 128],
                                        src_tile[:, o:o + 128], ident)
                if evac == "scalar":
                    nc.scalar.copy(dst[:, :], ps[:, :])
                else:
                    nc.vector.tensor_copy(dst[:, :], ps[:, :])

            memT = []
            for m in range(M):
                row = []
                for t in range(2):
                    mt = big.tile([128, GC], BF, tag=f"memT_{m}_{t}",
                                  name=f"memT_{m}_{t}")
                    transpose_in(mn, M * D, m * D + t * 128, mt, "scalar")
                    row.append(mt)
                memT.append(row)
            xT = []
            for t in range(2):
                xt = act.tile([128, GC], BF, tag=f"xT{t}", bufs=2,
                              name=f"xT{t}")
                transpose_in(xn, D, t * 128, xt, "scalar")
                xT.append(xt)
            hT, hTf = [], []
            for t in range(2):
                ps = ps_tile("ps_trh")
                for r4 in range(NR4):
                    o = r4 * D + t * 128
                    nc.tensor.transpose(ps[:, r4 * 128:(r4 + 1) * 128],
                                        hn[:, o:o + 128], ident)
                ht = act.tile([128, GC], BF, tag=f"hT{t}", bufs=2,
                              name=f"hT{t}")
                nc.scalar.copy(ht[:, :], ps[:, :])
                htf = act.tile([128, GC], FP32, tag=f"hTf{t}", name=f"hTf{t}")
                nc.vector.tensor_copy(htf[:, :], ps[:, :])
                hT.append(ht)
                hTf.append(htf)
            s.update(memT=memT, xT=xT, hT=hT, hTf=hTf)

        def mm_pair(lhs_tiles, out_slice, rhs_tiles, psum_tile,
                    start=True, stop=True):
            nkt = len(lhs_tiles)
            for kt in range(nkt):
                nc.tensor.matmul(
                    psum_tile[:, :], lhs_tiles[kt][:, out_slice],
                    rhs_tiles[kt][:, :],
                    start=(start and kt == 0), stop=(stop and kt == nkt - 1))

        def phase_GRU(g):
            s = ST[g]
            xT, hT, hTf = s["xT"], s["hT"], s["hTf"]
            zr = []
            for mt in range(4):
                ps = ps_tile(f"ps_zr{mt}")
                sl = slice(mt * 128, (mt + 1) * 128)
                mm_pair(W["w_zr_x"], sl, xT, ps, start=True, stop=False)
                mm_pair(W["w_zr_h"], sl, hT, ps, start=False, stop=True)
                zr_t = act.tile([128, GC], FP32, tag=f"zr{mt}", name=f"zr{mt}")
                nc.scalar.activation(zr_t[:, :], ps[:, :], ACTF.Sigmoid,
                                     bias=bzr_sb[mt][:, :], scale=1.0)
                zr.append(zr_t)
            z_f, r_f = zr[0:2], zr[2:4]

            hc_f, gru_f, gru_b16 = [], [], []
            for mt in range(2):
                sl = slice(mt * 128, (mt + 1) * 128)
                ps_xh = ps_tile(f"ps_xh{mt}")
                mm_pair(W["w_h_x"], sl, xT, ps_xh)
                ps_hh = ps_tile(f"ps_hh{mt}")
                mm_pair(W["w_hh"], sl, hT, ps_hh)
                t1 = act.tile([128, GC], FP32, tag="fscr", bufs=3,
                              name=f"t1_{mt}")
                nc.vector.scalar_tensor_tensor(t1[:, :], ps_hh[:, :],
                                               vbias("b_hh", mt),
                                               r_f[mt][:, :],
                                               op0=ALU.add, op1=ALU.mult)
                t2 = act.tile([128, GC], FP32, tag="fscr", bufs=3,
                              name=f"t2_{mt}")
                nc.vector.tensor_tensor(t2[:, :], t1[:, :], ps_xh[:, :],
                                        ALU.add)
                hc = act.tile([128, GC], FP32, tag=f"hc{mt}", name=f"hc{mt}")
                nc.scalar.activation(hc[:, :], t2[:, :], ACTF.Tanh,
                                     bias=vbias("b_xh", mt), scale=1.0)
                hc_f.append(hc)
                d1 = act.tile([128, GC], FP32, tag="fscr", bufs=3,
                              name=f"d1_{mt}")
                nc.gpsimd.tensor_tensor(d1[:, :], hTf[mt][:, :], hc[:, :],
                                        ALU.subtract)
                e1 = act.tile([128, GC], FP32, tag="fscr", bufs=3,
                              name=f"e1_{mt}")
                nc.gpsimd.tensor_tensor(e1[:, :], z_f[mt][:, :], d1[:, :],
                                        ALU.mult)
                gr = act.tile([128, GC], FP32, tag=f"gru{mt}", name=f"gru{mt}")
                nc.gpsimd.tensor_tensor(gr[:, :], hc[:, :], e1[:, :], ALU.add)
                gru_f.append(gr)
                grb = act.tile([128, GC], BF, tag=f"grub{mt}", name=f"grub{mt}")
                nc.vector.tensor_copy(grb[:, :], gr[:, :])
                gru_b16.append(grb)

            q_sb = []
            for mt in range(2):
                sl = slice(mt * 128, (mt + 1) * 128)
                ps = ps_tile(f"ps_q{mt}")
                mm_pair(W["w_q"], sl, gru_b16, ps)
                qs = act.tile([128, GC], BF, tag=f"q{mt}", name=f"q{mt}")
                nc.scalar.activation(qs[:, :], ps[:, :], ACTF.Identity,
                                     bias=vbias("bq_s", mt),
                                     scale=float(1.0 / np.sqrt(DK)))
                q_sb.append(qs)
            s.update(gru_f=gru_f, gru_b16=gru_b16, q_sb=q_sb)

        def phase_L1(g):
            s = ST[g]
            memT, q_sb = s["memT"], s["q_sb"]
            ps_sc = pst.tile([H * M, GC], FP32, tag="psc", bufs=1,
                             name="ps_scores")
            for m in range(M):
                for t in range(2):
                    sl = slice(t * 128, (t + 1) * 128)
                    ps_k = ps_tile(f"ps_k{m}{t}")
                    mm_pair(W["w_k"], sl, memT[m], ps_k)
                    prod = act.tile([128, GC], BF, tag=f"sprod{t}",
                                    name=f"sprod{m}{t}")
                    nc.vector.tensor_tensor(prod[:, :], q_sb[t][:, :],
                                            ps_k[:, :], ALU.mult)
                    nc.tensor.matmul(ps_sc[:, :], smask_sb[m][t],
                                     prod[:, :],
                                     start=(m == 0 and t == 0),
                                     stop=(m == M - 1 and t == 1))
            s["ps_sc"] = ps_sc

        def phase_SM(g):
            s = ST[g]
            ps_sc, xT = s["ps_sc"], s["xT"]
            e_sb = act.tile([H * M, GC], BF, tag="e_sb", name="e_sb")
            nc.scalar.activation(e_sb[:, :], ps_sc[:, :], ACTF.Exp)
            ps_sum = ps_tile("ps_sum", [H, GC])
            nc.tensor.matmul(ps_sum[:, :], summask_sb[:, :], e_sb[:, :],
                             start=True, stop=True)
            rec_f = sml.tile([H, GC], FP32, tag="lnscr", bufs=2, name="rec_f")
            nc.vector.reciprocal_approx_fast(rec_f[:, :], ps_sum[:, :])
            rec_sb = sml.tile([H, GC], BF, tag="rec", name="rec_sb")
            nc.vector.tensor_copy(rec_sb[:, :], rec_f[:, :])
            ps_rr = ps_tile("ps_rr", [H * M, GC])
            nc.tensor.matmul(ps_rr[:, :], replmask_sb[:, :], rec_sb[:, :],
                             start=True, stop=True)
            attn_sb = act.tile([H * M, GC], BF, tag="attn", name="attn_sb")
            nc.vector.tensor_tensor(attn_sb[:, :], e_sb[:, :], ps_rr[:, :],
                                    ALU.mult)
            # new_entry here: PE filler during softmax
            neT = []
            for mt in range(2):
                sl = slice(mt * 128, (mt + 1) * 128)
                ps = ps_tile(f"ps_ne{mt}")
                mm_pair(W["w_ne"], sl, xT, ps)
                ne = act.tile([128, GC], FP32, tag=f"neT{mt}", name=f"neT{mt}")
                nc.scalar.activation(ne[:, :], ps[:, :], ACTF.Identity,
                                     bias=vbias("b_ne", mt), scale=1.0)
                neT.append(ne)
            s.update(attn_sb=attn_sb, neT=neT)

        def phase_L2(g):
            s = ST[g]
            memT, attn_sb = s["memT"], s["attn_sb"]
            MC = 2 * GC
            slab = big.tile([128, 5 * MC], BF, tag="pslab", name="pslab")
            for m in range(M):
                tmp = None
                if m >= 5:
                    tmp = big.tile([128, MC], BF, tag="ptmp", bufs=2,
                                   name=f"ptmp{m}")
                for t in range(2):
                    sl = slice(t * 128, (t + 1) * 128)
                    ps_v = ps_tile(f"ps_v{m}{t}")
                    mm_pair(W["w_v"], sl, memT[m], ps_v)
                    ps_er = ps_tile(f"ps_er{m}{t}")
                    nc.tensor.matmul(ps_er[:, :], amask_sb[m][t],
                                     attn_sb[:, :], start=True, stop=True)
                    er_sb = sml.tile([128, GC], BF, tag="er", bufs=1,
                                     name=f"er{m}{t}")
                    nc.scalar.copy(er_sb[:, :], ps_er[:, :])
                    dst = (slab[:, m * MC + t * GC:m * MC + (t + 1) * GC]
                           if m < 5 else tmp[:, t * GC:(t + 1) * GC])
                    nc.vector.tensor_tensor(dst, er_sb[:, :], ps_v[:, :],
                                            ALU.mult)
                if m >= 5:
                    c = (m - 5) * MC
                    eng = nc.gpsimd if (m % 2) else nc.vector
                    eng.tensor_tensor(slab[:, c:c + MC], slab[:, c:c + MC],
                                      tmp[:, :], ALU.add)
            nc.gpsimd.tensor_tensor(slab[:, 0:2 * MC], slab[:, 0:2 * MC],
                                    slab[:, 2 * MC:4 * MC], ALU.add)
            nc.vector.tensor_tensor(slab[:, 0:MC], slab[:, 0:MC],
                                    slab[:, MC:2 * MC], ALU.add)
            nc.gpsimd.tensor_tensor(slab[:, 0:MC], slab[:, 0:MC],
                                    slab[:, 4 * MC:5 * MC], ALU.add)
            U_sb = []
            for t in range(2):
                u_t = act.tile([128, GC], BF, tag=f"U{t}", name=f"U{t}")
                nc.vector.tensor_copy(u_t[:, :], slab[:, t * GC:(t + 1) * GC])
                U_sb.append(u_t)
            s["U_sb"] = U_sb

        def ln_stats(x_b16_tiles, sq_tag):
            ps_s1 = ps_tile("ps_s1_" + sq_tag, [1, GC])
            for kt in range(2):
                nc.tensor.matmul(ps_s1[:, :], ones_128_1[:, :],
                                 x_b16_tiles[kt][:, :],
                                 start=(kt == 0), stop=(kt == 1))
            sq = [act.tile([128, GC], BF, tag=f"sq{kt}",
                           name=f"{sq_tag}{kt}") for kt in range(2)]
            for kt in range(2):
                nc.vector.tensor_tensor(sq[kt][:, :], x_b16_tiles[kt][:, :],
                                        x_b16_tiles[kt][:, :], ALU.mult)
            ps_s2 = ps_tile("ps_s2_" + sq_tag, [1, GC])
            for kt in range(2):
                nc.tensor.matmul(ps_s2[:, :], ones_128_1[:, :],
                                 sq[kt][:, :], start=(kt == 0), stop=(kt == 1))
            s1sq = sml.tile([1, GC], FP32, tag="lnscr", bufs=2,
                            name="s1sq_" + sq_tag)
            nc.scalar.activation(s1sq[:, :], ps_s1[:, :], ACTF.Square)
            var_t = sml.tile([1, GC], FP32, tag="lnscr", bufs=2,
                             name="var_" + sq_tag)
            nc.vector.scalar_tensor_tensor(var_t[:, :], ps_s2[:, :],
                                           float(D), s1sq[:, :],
                                           op0=ALU.mult, op1=ALU.subtract)
            sd = sml.tile([1, GC], FP32, tag="lnscr", bufs=2,
                          name="sd_" + sq_tag)
            nc.scalar.activation(sd[:, :], var_t[:, :], ACTF.Sqrt,
                                 bias=vecs_sb[0][0:1, VEC["eps_dd"]:
                                                 VEC["eps_dd"] + 1],
                                 scale=1.0)
            rc = sml.tile([1, GC], FP32, tag="lnscr", bufs=2,
                          name="rc_" + sq_tag)
            nc.vector.reciprocal_approx_fast(rc[:, :], sd[:, :])
            A_b = sml.tile([1, GC], BF, tag="A_b", name="A_" + sq_tag)
            with nc.allow_low_precision("LN scale bf16"):
                nc.vector.tensor_scalar(A_b[:, :], rc[:, :], float(D), None,
                                        op0=ALU.mult)
            B_b = sml.tile([1, GC], BF, tag="B_b", name="B_" + sq_tag)
            nc.vector.tensor_tensor(B_b[:, :], ps_s1[:, :], rc[:, :], ALU.mult)
            return A_b, B_b, ps_s1

        def phase_TAILA(g):
            s = ST[g]
            U_sb = s["U_sb"]
            ctx_b16 = []
            for mt in range(2):
                sl = slice(mt * 128, (mt + 1) * 128)
                ps = ps_tile(f"ps_ctx{mt}")
                for kt in range(2):
                    nc.tensor.matmul(ps[:, :], W["w_o"][kt][:, sl],
                                     U_sb[kt][:, :],
                                     start=(kt == 0), stop=(kt == 1))
                cb = act.tile([128, GC], BF, tag=f"ctxb{mt}", name=f"ctxb{mt}")
                nc.scalar.activation(cb[:, :], ps[:, :], ACTF.Identity,
                                     bias=vbias("bo_p", mt), scale=1.0)
                ctx_b16.append(cb)

            A1, B1, ps_s1_ln1 = ln_stats(ctx_b16, "sqc")
            s1_b16 = sml.tile([1, GC], BF, tag="s1b", name="s1_b16")
            nc.vector.tensor_copy(s1_b16[:, :], ps_s1_ln1[:, :])
            ps_A1 = ps_tile("ps_A1rep")
            nc.tensor.matmul(ps_A1[:, :], ones_1_128[:, :], A1[:, :],
                             start=True, stop=True)
            A1rep = act.tile([128, GC], BF, tag="A1rep", name="A1rep")
            nc.scalar.copy(A1rep[:, :], ps_A1[:, :])

            ctxp_f, ctxp_b16 = [], []
            for mt in range(2):
                sl = slice(mt * 128, (mt + 1) * 128)
                ps = ps_tile(f"ps_cp{mt}")
                for kt in range(2):
                    nc.tensor.matmul(ps[:, :], W["w_ctx"][kt][:, sl],
                                     ctx_b16[kt][:, :],
                                     start=(kt == 0), stop=False)
                nc.tensor.matmul(ps[:, :], W["neg_gw"][:, sl], s1_b16[:, :],
                                 start=False, stop=True)
                tmul = act.tile([128, GC], BF, tag=f"cpm{mt}", name=f"cpm{mt}")
                nc.vector.tensor_tensor(tmul[:, :], A1rep[:, :], ps[:, :],
                                        ALU.mult)
                cpf = act.tile([128, GC], FP32, tag=f"ctxp{mt}",
                               name=f"ctxp{mt}")
                nc.scalar.activation(cpf[:, :], tmul[:, :], ACTF.Tanh,
                                     bias=vbias("bw_ctx", mt), scale=1.0)
                ctxp_f.append(cpf)
                cpb = act.tile([128, GC], BF, tag=f"ctxpb{mt}",
                               name=f"ctxpb{mt}")
                nc.vector.tensor_copy(cpb[:, :], cpf[:, :])
                ctxp_b16.append(cpb)
            s.update(ctxp_f=ctxp_f, ctxp_b16=ctxp_b16)

        def phase_TAILB(g):
            s = ST[g]
            gru_f, gru_b16 = s["gru_f"], s["gru_b16"]
            ctxp_f, ctxp_b16 = s["ctxp_f"], s["ctxp_b16"]
            neT = s["neT"]
            rows = slice(g * GC, (g + 1) * GC)
            alpha_f = []
            for mt in range(2):
                sl = slice(mt * 128, (mt + 1) * 128)
                ps = ps_tile(f"ps_al{mt}")
                for kt in range(2):
                    nc.tensor.matmul(ps[:, :], W["w_gate"][kt][:, sl],
                                     gru_b16[kt][:, :],
                                     start=(kt == 0), stop=False)
                for kt in range(2):
                    nc.tensor.matmul(ps[:, :], W["w_gate"][2 + kt][:, sl],
                                     ctxp_b16[kt][:, :],
                                     start=False, stop=(kt == 1))
                al = act.tile([128, GC], FP32, tag=f"alpha{mt}",
                              name=f"alpha{mt}")
                nc.scalar.activation(al[:, :], ps[:, :], ACTF.Sigmoid,
                                     bias=vbias("b_gate", mt), scale=1.0)
                alpha_f.append(al)

            blend_f, blend_b16 = [], []
            for mt in range(2):
                d2 = act.tile([128, GC], FP32, tag="fscr", bufs=3,
                              name=f"d2_{mt}")
                nc.gpsimd.tensor_tensor(d2[:, :], ctxp_f[mt][:, :],
                                        gru_f[mt][:, :], ALU.subtract)
                e2 = act.tile([128, GC], FP32, tag="fscr", bufs=3,
                              name=f"e2_{mt}")
                nc.gpsimd.tensor_tensor(e2[:, :], alpha_f[mt][:, :], d2[:, :],
                                        ALU.mult)
                bl = act.tile([128, GC], FP32, tag=f"blend{mt}",
                              name=f"blend{mt}")
                nc.gpsimd.tensor_tensor(bl[:, :], gru_f[mt][:, :], e2[:, :],
                                        ALU.add)
                blend_f.append(bl)
                bb = act.tile([128, GC], BF, tag=f"blendb{mt}",
                              name=f"blendb{mt}")
                nc.vector.tensor_copy(bb[:, :], bl[:, :])
                blend_b16.append(bb)

            A2, B2, _ = ln_stats(blend_b16, "sqb")
            hcT = []
            for mt in range(2):
                sl = slice(mt * 128, (mt + 1) * 128)
                ps_Ag = ps_tile(f"ps_Ag{mt}")
                nc.tensor.matmul(ps_Ag[:, :], W["g_out_row"][:, sl], A2[:, :],
                                 start=True, stop=True)
                ps_Bg = ps_tile(f"ps_Bg{mt}")
                nc.tensor.matmul(ps_Bg[:, :], W["g_out_row"][:, sl], B2[:, :],
                                 start=True, stop=False)
                nc.tensor.matmul(ps_Bg[:, :], W["neg_beta_out_row"][:, sl],
                                 ones_row[:, :], start=False, stop=True)
                t3 = act.tile([128, GC], FP32, tag="fscr", bufs=3,
                              name=f"t3_{mt}")
                nc.vector.tensor_tensor(t3[:, :], blend_f[mt][:, :],
                                        ps_Ag[:, :], ALU.mult)
                hct = act.tile([128, GC], FP32, tag=f"hcT{mt}", name=f"hcT{mt}")
                nc.vector.tensor_tensor(hct[:, :], t3[:, :], ps_Bg[:, :],
                                        ALU.subtract)
                hcT.append(hct)

            hc_nat = sml.tile([128, NR4 * D], FP32, tag="hcnat", bufs=1,
                              name="hc_nat")
            ne_nat = sml.tile([128, NR4 * D], FP32, tag="nenat", bufs=1,
                              name="ne_nat")
            for r4 in range(NR4):
                ps = ps_tile(f"ps_otr{r4}", [128, D])
                for mt in range(2):
                    nc.tensor.transpose(ps[:, mt * 128:(mt + 1) * 128],
                                        hcT[mt][:, r4 * 128:(r4 + 1) * 128],
                                        ident)
                nc.vector.tensor_copy(hc_nat[:, r4 * D:(r4 + 1) * D],
                                      ps[:, :])
                ps2 = ps_tile(f"ps_otr2{r4}", [128, D])
                for mt in range(2):
                    nc.tensor.transpose(ps2[:, mt * 128:(mt + 1) * 128],
                                        neT[mt][:, r4 * 128:(r4 + 1) * 128],
                                        ident)
                nc.vector.tensor_copy(ne_nat[:, r4 * D:(r4 + 1) * D],
                                      ps2[:, :])
            nc.scalar.dma_start(
                hc_d[rows, :].rearrange("(a p) c -> p a c", p=128),
                hc_nat[:, :])
            nc.scalar.dma_start(
                nm_d[rows, (M - 1) * D:M * D].rearrange("(a p) c -> p a c",
                                                        p=128),
                ne_nat[:, :])

        # interleaved emission: group 1's PE-heavy phases fill group 0's
        # DVE/ACT-heavy tail gaps (and vice versa)
        phase_T(0)
        phase_GRU(0)
        phase_L1(0)
        phase_SM(0)
        phase_L2(0)
        phase_T(1)
        phase_TAILA(0)
        phase_TAILB(0)
        phase_GRU(1)
        phase_L1(1)
        phase_SM(1)
        phase_L2(1)
        phase_TAILA(1)
        phase_TAILB(1)

    nc.compile()
    return nc


def _get_kernel():
    if "nc" not in _BUILD_CACHE:
        _BUILD_CACHE["nc"] = build_kernel()
    return _BUILD_CACHE["nc"]


def make_in_maps(inputs):
    w = _prep_weights(inputs)
    x = np.ascontiguousarray(np.asarray(inputs["inputs"], np.float32))
    h = np.ascontiguousarray(np.asarray(inputs["h_prev"], np.float32))
    mem = np.ascontiguousarray(np.asarray(inputs["memory_flat"], np.float32))
    in_maps = []
    for c in range(N_CORES):
        s = slice(c * BS, (c + 1) * BS)
        in_maps.append({"x": x[s], "h": h[s], "mem": mem[s],
                        "kb": w["kb"], "kf": w["kf"]})
    return in_maps


def kernel(**inputs):
    nc = _get_kernel()
    in_maps = make_in_maps(inputs)
    res = run_bass_kernel_spmd(nc, in_maps, core_ids=list(range(N_CORES)))
    h_corr = np.concatenate([res.results[c]["hcorr"] for c in range(N_CORES)],
                            axis=0)
    new_mem = np.concatenate([res.results[c]["newmem"] for c in range(N_CORES)],
                             axis=0)
    return h_corr, new_mem


# revision 24
# speedup vs baseline: 1.2184x; 1.0534x over previous
"""Trainium2 Bass kernel for nn_MATAPCell (GRU + single-query MHA over per-row
memory + gated blend + memory shift-write).

Contract: kernel(**inputs) takes FULL unsharded fp32 inputs (see shapes below),
shards batch across 8 NeuronCores (pure data parallel, weights replicated),
runs a Bass/Tile kernel per core, and gathers the full outputs.

Returns (h_corr [B,256] f32, new_memory_flat [B,2560] f32) matching reference.

Dataflow per core (BS=1024 rows, two column-groups of 512):
  - activations live TRANSPOSED in SBUF: [feature(partitions), row(free)]
  - natural-layout rows are DMA'd in and PE-transposed (f32) into bf16 tiles
  - all matmuls bf16 (weights host-cast), fp32 PSUM accumulate
  - attention: k = mem@Wk on PE; scores q.k via DVE mult + PE mask-matmul
    partition reduction; softmax (no max-subtract: |scores| < 0.3);
    apply: v on PE, attn replicated across dk-partitions via PE mask-matmul,
    DVE mult, wide-AP tree accumulation
  - h-path carrier (h_prev -> gru_out -> blend -> LN2 -> h_corr) kept f32
    in elementwise ops for accuracy
  - memory shift is a pure SBUF->DRAM f32 copy of the loaded mem tile
  - layernorms computed in T-layout via ones-matmul partition sums + rank-1
    replication matmuls
"""

import sys
import os
import numpy as np

for _p in ("/opt/trn_rl_repo",):
    if os.path.isdir(_p) and _p not in sys.path:
        sys.path.insert(0, _p)

import ml_dtypes
from contextlib import ExitStack

import concourse.bass as bass
import concourse.tile as tile
from concourse import bacc, mybir
from concourse.bass_utils import run_bass_kernel_spmd

BF16 = ml_dtypes.bfloat16
FP32 = mybir.dt.float32
BF = mybir.dt.bfloat16

B, D, M, H, DK = 8192, 256, 10, 4, 64
N_CORES = 8
BS = B // N_CORES            # 1024 rows per core
NG = BS // 512               # column groups per core (512 cols each)
GC = 512                     # cols per group
NR4 = 4                      # row-tiles (128 rows) per group
HKD = H * DK                 # 256
D3 = 3 * D                   # 768
LN_EPS = 1e-3
ALU = mybir.AluOpType
ACTF = mybir.ActivationFunctionType

_BUILD_CACHE = {}


def _blob_layout():
    """(name, rows, cols) entries for the bf16 and f32 constant blobs."""
    b = []
    for nm, cols in [("w_zr_x", 512), ("w_zr_h", 512), ("w_h_x", 256),
                     ("w_hh", 256), ("w_ne", 256), ("w_q", 256), ("w_k", 256),
                     ("w_v", 256), ("w_o", 256), ("w_ctx", 256)]:
        b.append((nm + "_0", 128, cols))
        b.append((nm + "_1", 128, cols))
    for k in range(4):
        b.append((f"w_gate_{k}", 128, 256))
    b += [("neg_gw", 1, 256), ("g_out_row", 1, 256),
          ("neg_beta_out_row", 1, 256), ("ones_row", 1, 512),
          ("ones_1_128", 1, 128), ("ones_128_1", 128, 1),
          ("summask", 40, 4), ("replmask", 4, 40)]
    for m in range(M):
        for t in range(2):
            b.append((f"smask_{m}_{t}", 128, 40))
            b.append((f"amask_{m}_{t}", 40, 128))
    f = [("vecs_0", 128, NVEC), ("vecs_1", 128, NVEC)]
    for k in range(4):
        f.append((f"bzr_{k}", 128, 1))
    f.append(("ident", 128, 128))
    return b, f


def _blob_offsets():
    b, f = _blob_layout()
    bo, off = {}, 0
    for nm, rows, cols in b:
        bo[nm] = (off, rows, cols)
        off += cols
    bcols = off
    fo, off = {}, 0
    for nm, rows, cols in f:
        fo[nm] = (off, rows, cols)
        off += cols
    return bo, bcols, fo, off


VEC = {"b_xh2": 0, "b_hh": 1, "bq_s": 2, "bo_p": 3, "bw_ctx2": 4,
       "b_gate": 5, "b_ne": 6, "eps_dd": 7}
NVEC = 8
BOFF, BCOLS, FOFF, FCOLS = _blob_offsets()


def _prep_weights(inp):
    """Host-side weight fusion + bf16 casts + blob packing. All small."""
    f = lambda x: np.asarray(x, np.float32)
    W_in = f(inp["W_in"]); b_in = f(inp["b_in"])
    gru_k = f(inp["gru_k"]); gru_rk = f(inp["gru_rk"]); gru_b = f(inp["gru_b"])
    Wq = f(inp["Wq"]).reshape(D, HKD); bq = f(inp["bq"]).reshape(HKD)
    Wk = f(inp["Wk"]).reshape(D, HKD)
    Wv = f(inp["Wv"]).reshape(D, HKD); bv = f(inp["bv"]).reshape(HKD)
    Wo = f(inp["Wo"]).reshape(HKD, D); bo = f(inp["bo"])
    g_attn = f(inp["g_attn"]); beta_attn = f(inp["beta_attn"])
    g_out = f(inp["g_out"]); beta_out = f(inp["beta_out"])
    W_ctx = f(inp["W_ctx"]); b_ctx = f(inp["b_ctx"])
    W_gate = f(inp["W_gate"]); b_gate = f(inp["b_gate"])
    W_mem = f(inp["W_mem"]); b_mem = f(inp["b_mem"])

    t = {}
    for nm, arr in [("w_zr_x", W_in @ gru_k[:, :2 * D]),
                    ("w_zr_h", gru_rk[:, :2 * D]),
                    ("w_h_x", W_in @ gru_k[:, 2 * D:]),
                    ("w_hh", gru_rk[:, 2 * D:]),
                    ("w_ne", W_in @ W_mem), ("w_q", Wq), ("w_k", Wk),
                    ("w_v", Wv), ("w_o", Wo),
                    ("w_ctx", g_attn[:, None] * W_ctx)]:
        t[nm + "_0"] = arr[0:128]
        t[nm + "_1"] = arr[128:256]
    for k in range(4):
        t[f"w_gate_{k}"] = W_gate[k * 128:(k + 1) * 128]
    gw = g_attn @ W_ctx
    t["neg_gw"] = -(gw / np.float32(D))[None, :]
    t["g_out_row"] = g_out[None, :]
    t["neg_beta_out_row"] = -beta_out[None, :]
    t["ones_row"] = np.ones((1, 512), np.float32)
    t["ones_1_128"] = np.ones((1, 128), np.float32)
    t["ones_128_1"] = np.ones((128, 1), np.float32)
    summask = np.zeros((H * M, H), np.float32)
    for h in range(H):
        summask[h * M:(h + 1) * M, h] = 1.0
    t["summask"] = summask
    repl = np.zeros((H, H * M), np.float32)
    for h in range(H):
        repl[h, h * M:(h + 1) * M] = 1.0
    t["replmask"] = repl
    for m in range(M):
        for tt in range(2):
            sm = np.zeros((128, H * M), np.float32)
            am = np.zeros((H * M, 128), np.float32)
            for p in range(128):
                h = 2 * tt + p // DK
                sm[p, h * M + m] = 1.0
                am[h * M + m, p] = 1.0
            t[f"smask_{m}_{tt}"] = sm
            t[f"amask_{m}_{tt}"] = am

    kb = np.zeros((128, BCOLS), np.float32)
    for nm, (off, rows, cols) in BOFF.items():
        kb[:rows, off:off + cols] = t[nm]
    kb = kb.astype(BF16)

    b_zr = gru_b[0, :2 * D] + gru_b[1, :2 * D] + b_in @ gru_k[:, :2 * D]
    b_xh = gru_b[0, 2 * D:] + b_in @ gru_k[:, 2 * D:]
    b_hh = gru_b[1, 2 * D:]
    bq_s = bq / np.float32(np.sqrt(DK))
    bo_p = bv @ Wo + bo
    bw_ctx = beta_attn @ W_ctx + b_ctx
    b_ne = b_in @ W_mem + b_mem
    eps_col = np.full((D,), np.float32(D) * np.float32(D) * np.float32(LN_EPS),
                      np.float32)
    vecs = np.stack([2.0 * b_xh, b_hh, bq_s, bo_p, 2.0 * bw_ctx, b_gate,
                     b_ne, eps_col], axis=1)
    tf = {"vecs_0": vecs[0:128], "vecs_1": vecs[128:256],
          "ident": np.eye(128, dtype=np.float32)}
    for k in range(4):
        tf[f"bzr_{k}"] = b_zr[k * 128:(k + 1) * 128, None]
    kf = np.zeros((128, FCOLS), np.float32)
    for nm, (off, rows, cols) in FOFF.items():
        kf[:rows, off:off + cols] = tf[nm]
    return {"kb": np.ascontiguousarray(kb), "kf": np.ascontiguousarray(kf)}


def build_kernel():
    nc = bacc.Bacc("TRN2", target_bir_lowering=False, debug=False,
                   num_devices=N_CORES)

    def din(name, shape, dt=FP32):
        return nc.dram_tensor(name, shape, dt, kind="ExternalInput").ap()

    def dout(name, shape, dt=FP32):
        return nc.dram_tensor(name, shape, dt, kind="ExternalOutput").ap()

    x_d = din("x", (BS, D))
    h_d = din("h", (BS, D))
    mem_d = din("mem", (BS, M * D))
    kb_d = din("kb", (128, BCOLS), BF)
    kf_d = din("kf", (128, FCOLS))
    hc_d = dout("hcorr", (BS, D))
    nm_d = dout("newmem", (BS, M * D))

    with tile.TileContext(nc) as tc, ExitStack() as ctx:
        konst = ctx.enter_context(tc.tile_pool(name="konst", bufs=1))
        nat = ctx.enter_context(tc.tile_pool(name="nat", bufs=1))
        big = ctx.enter_context(tc.tile_pool(name="big", bufs=1))
        act = ctx.enter_context(tc.tile_pool(name="act", bufs=1))
        sml = ctx.enter_context(tc.tile_pool(name="sml", bufs=1))
        pst = ctx.enter_context(tc.tile_pool(name="pst", bufs=1, space="PSUM"))

        def ps_tile(name, shape=None, tag="ps", bufs=3):
            return pst.tile(shape or [128, GC], FP32, tag=tag, bufs=bufs,
                            name=name)

        def pml_tile(name):
            return ps_tile(name, tag="pml", bufs=4)

        # ---- constant blobs: 2 DMAs total (kf first: holds the transpose
        # identity; weights (kb) aren't needed until GRU) ----
        kf_sb = konst.tile([128, FCOLS], FP32, tag="kf", name="kf_sb")
        nc.sync.dma_start(kf_sb[:, :], kf_d[:, :])
        kb_sb = konst.tile([128, BCOLS], BF, tag="kb", name="kb_sb")

        def KB(nm):
            off, rows, cols = BOFF[nm]
            return kb_sb[0:rows, off:off + cols]

        def KF(nm):
            off, rows, cols = FOFF[nm]
            return kf_sb[0:rows, off:off + cols]

        W = {nm: [KB(nm + "_0"), KB(nm + "_1")]
             for nm in ("w_zr_x", "w_zr_h", "w_h_x", "w_hh", "w_ne", "w_q",
                        "w_k", "w_v", "w_o", "w_ctx")}
        W["w_gate"] = [KB(f"w_gate_{k}") for k in range(4)]
        smask_sb = [[KB(f"smask_{m}_{t}") for t in range(2)] for m in range(M)]
        amask_sb = [[KB(f"amask_{m}_{t}") for t in range(2)] for m in range(M)]
        summask_sb = KB("summask")
        replmask_sb = KB("replmask")
        ones_128_1 = KB("ones_128_1")
        ones_1_128 = KB("ones_1_128")
        ones_row = KB("ones_row")
        vecs_sb = [KF("vecs_0"), KF("vecs_1")]
        bzr_sb = [KF(f"bzr_{k}") for k in range(4)]
        ident = KF("ident")
        W["neg_gw"] = KB("neg_gw")
        W["g_out_row"] = KB("g_out_row")
        W["neg_beta_out_row"] = KB("neg_beta_out_row")

        def vbias(col, kt):
            return vecs_sb[kt][:, VEC[col]:VEC[col] + 1]

        # ---------------- per-group pipeline, phase-interleaved ----------------
        ST = [dict() for _ in range(NG)]

        def preload(g):
            s = ST[g]
            rows = slice(g * GC, (g + 1) * GC)
            xn = nat.tile([128, NR4 * D], FP32, tag="xnat", bufs=2, name="xn")
            nc.sync.dma_start(
                xn[:, :], x_d[rows, :].rearrange("(a p) c -> p a c", p=128))
            hn = nat.tile([128, NR4 * D], FP32, tag="hnat", bufs=2, name="hn")
            nc.sync.dma_start(
                hn[:, :], h_d[rows, :].rearrange("(a p) c -> p a c", p=128))
            mn = nat.tile([128, NR4 * M * D], FP32, tag="memnat", name="mn")
            for r4 in range(NR4):
                r = NR4 * g + r4
                nc.sync.dma_start(mn[:, r4 * M * D:(r4 + 1) * M * D],
                                  mem_d[r * 128:(r + 1) * 128, :])
            # memory shift store (scalar HWDGE queue; won't stall loads)
            nc.scalar.dma_start(
                nm_d[rows, 0:(M - 1) * D].rearrange("(a p) c -> p a c", p=128),
                mn.rearrange("p (a c) -> p a c", a=NR4)[:, :, D:M * D])
            s.update(xn=xn, hn=hn, mn=mn)

        def phase_T(g):
            s = ST[g]
            xn, hn, mn = s["xn"], s["hn"], s["mn"]

            def transpose_in(src_tile, stride, col_off, dst, evac):
                ps = ps_tile("ps_tr")
                for r4 in range(NR4):
                    o = r4 * stride + col_off
                    nc.tensor.transpose(ps[:, r4 * 128:(r4 + 1) * 128],
                                        src_tile[:, o:o + 128], ident)
                if evac == "scalar":
                    nc.scalar.copy(dst[:, :], ps[:, :])
                else:
                    nc.vector.tensor_copy(dst[:, :], ps[:, :])

            memT = []
            for m in range(M):
                row = []
                for t in range(2):
                    mt = big.tile([128, GC], BF, tag=f"memT_{m}_{t}",
                                  name=f"memT_{m}_{t}")
                    transpose_in(mn, M * D, m * D + t * 128, mt, "scalar")
                    row.append(mt)
                memT.append(row)
            xT = []
            for t in range(2):
                xt = act.tile([128, GC], BF, tag=f"xT{t}", bufs=2,
                              name=f"xT{t}")
                transpose_in(xn, D, t * 128, xt, "scalar")
                xT.append(xt)
            hT, hTf = [], []
            for t in range(2):
                ps = ps_tile("ps_trh")
                for r4 in range(NR4):
                    o = r4 * D + t * 128
                    nc.tensor.transpose(ps[:, r4 * 128:(r4 + 1) * 128],
                                        hn[:, o:o + 128], ident)
                ht = act.tile([128, GC], BF, tag=f"hT{t}", bufs=2,
                              name=f"hT{t}")
                nc.scalar.copy(ht[:, :], ps[:, :])
                htf = act.tile([128, GC], FP32, tag=f"hTf{t}", name=f"hTf{t}")
                nc.vector.tensor_copy(htf[:, :], ps[:, :])
                hT.append(ht)
                hTf.append(htf)
            s.update(memT=memT, xT=xT, hT=hT, hTf=hTf)

        def mm_pair(lhs_tiles, out_slice, rhs_tiles, psum_tile,
                    start=True, stop=True):
            nkt = len(lhs_tiles)
            for kt in range(nkt):
                nc.tensor.matmul(
                    psum_tile[:, :], lhs_tiles[kt][:, out_slice],
                    rhs_tiles[kt][:, :],
                    start=(start and kt == 0), stop=(stop and kt == nkt - 1))

        def phase_GRU(g):
            s = ST[g]
            xT, hT, hTf = s["xT"], s["hT"], s["hTf"]
            zr = []
            for mt in range(4):
                ps = ps_tile(f"ps_zr{mt}")
                sl = slice(mt * 128, (mt + 1) * 128)
                mm_pair(W["w_zr_x"], sl, xT, ps, start=True, stop=False)
                mm_pair(W["w_zr_h"], sl, hT, ps, start=False, stop=True)
                zr_t = act.tile([128, GC], FP32, tag=f"zr{mt}", name=f"zr{mt}")
                nc.scalar.activation(zr_t[:, :], ps[:, :], ACTF.Sigmoid,
                                     bias=bzr_sb[mt][:, :], scale=1.0)
                zr.append(zr_t)
            z_f, r_f = zr[0:2], zr[2:4]

            hc_f, gru_f, gru_b16 = [], [], []
            for mt in range(2):
                sl = slice(mt * 128, (mt + 1) * 128)
                ps_xh = ps_tile(f"ps_xh{mt}")
                mm_pair(W["w_h_x"], sl, xT, ps_xh)
                ps_hh = ps_tile(f"ps_hh{mt}")
                mm_pair(W["w_hh"], sl, hT, ps_hh)
                t1 = act.tile([128, GC], FP32, tag="fscr", bufs=3,
                              name=f"t1_{mt}")
                nc.vector.scalar_tensor_tensor(t1[:, :], ps_hh[:, :],
                                               vbias("b_hh", mt),
                                               r_f[mt][:, :],
                                               op0=ALU.add, op1=ALU.mult)
                t2 = act.tile([128, GC], FP32, tag="fscr", bufs=3,
                              name=f"t2_{mt}")
                nc.vector.tensor_tensor(t2[:, :], t1[:, :], ps_xh[:, :],
                                        ALU.add)
                sgm = act.tile([128, GC], FP32, tag="fscr", bufs=3,
                               name=f"sgm{mt}")
                nc.scalar.activation(sgm[:, :], t2[:, :], ACTF.Sigmoid,
                                     bias=vbias("b_xh2", mt), scale=2.0)
                hc = act.tile([128, GC], FP32, tag=f"hc{mt}", name=f"hc{mt}")
                nc.vector.tensor_scalar(hc[:, :], sgm[:, :], 2.0, -1.0,
                                        op0=ALU.mult, op1=ALU.add)
                hc_f.append(hc)
                d1 = act.tile([128, GC], FP32, tag="fscr", bufs=3,
                              name=f"d1_{mt}")
                nc.vector.tensor_tensor(d1[:, :], hTf[mt][:, :], hc[:, :],
                                        ALU.subtract)
                e1 = act.tile([128, GC], FP32, tag="fscr", bufs=3,
                              name=f"e1_{mt}")
                nc.vector.tensor_tensor(e1[:, :], z_f[mt][:, :], d1[:, :],
                                        ALU.mult)
                gr = act.tile([128, GC], FP32, tag=f"gru{mt}", name=f"gru{mt}")
                nc.vector.tensor_tensor(gr[:, :], hc[:, :], e1[:, :], ALU.add)
                gru_f.append(gr)
                grb = act.tile([128, GC], BF, tag=f"grub{mt}", name=f"grub{mt}")
                nc.vector.tensor_copy(grb[:, :], gr[:, :])
                gru_b16.append(grb)

            q_sb = []
            for mt in range(2):
                sl = slice(mt * 128, (mt + 1) * 128)
                ps = ps_tile(f"ps_q{mt}")
                mm_pair(W["w_q"], sl, gru_b16, ps)
                qs = act.tile([128, GC], BF, tag=f"q{mt}", name=f"q{mt}")
                nc.scalar.activation(qs[:, :], ps[:, :], ACTF.Identity,
                                     bias=vbias("bq_s", mt),
                                     scale=float(1.0 / np.sqrt(DK)))
                q_sb.append(qs)
            s.update(gru_f=gru_f, gru_b16=gru_b16, q_sb=q_sb)

        def phase_L1(g):
            s = ST[g]
            memT, q_sb = s["memT"], s["q_sb"]
            ps_sc = pst.tile([H * M, GC], FP32, tag="psc", bufs=1,
                             name="ps_scores")
            for m in range(M):
                for t in range(2):
                    sl = slice(t * 128, (t + 1) * 128)
                    ps_k = pml_tile(f"ps_k{m}{t}")
                    mm_pair(W["w_k"], sl, memT[m], ps_k)
                    prod = act.tile([128, GC], BF, tag=f"sprod{t}",
                                    name=f"sprod{m}{t}")
                    nc.vector.tensor_tensor(prod[:, :], q_sb[t][:, :],
                                            ps_k[:, :], ALU.mult)
                    nc.tensor.matmul(ps_sc[:, :], smask_sb[m][t],
                                     prod[:, :],
                                     start=(m == 0 and t == 0),
                                     stop=(m == M - 1 and t == 1))
            s["ps_sc"] = ps_sc

        def phase_SM(g):
            s = ST[g]
            ps_sc, xT = s["ps_sc"], s["xT"]
            e_sb = act.tile([H * M, GC], BF, tag="e_sb", name="e_sb")
            nc.scalar.activation(e_sb[:, :], ps_sc[:, :], ACTF.Exp)
            ps_sum = ps_tile("ps_sum", [H, GC])
            nc.tensor.matmul(ps_sum[:, :], summask_sb[:, :], e_sb[:, :],
                             start=True, stop=True)
            rec_f = sml.tile([H, GC], FP32, tag="lnscr", bufs=2, name="rec_f")
            nc.vector.reciprocal_approx_fast(rec_f[:, :], ps_sum[:, :])
            rec_sb = sml.tile([H, GC], BF, tag="rec", name="rec_sb")
            nc.vector.tensor_copy(rec_sb[:, :], rec_f[:, :])
            ps_rr = ps_tile("ps_rr", [H * M, GC])
            nc.tensor.matmul(ps_rr[:, :], replmask_sb[:, :], rec_sb[:, :],
                             start=True, stop=True)
            attn_sb = act.tile([H * M, GC], BF, tag="attn", name="attn_sb")
            nc.vector.tensor_tensor(attn_sb[:, :], e_sb[:, :], ps_rr[:, :],
                                    ALU.mult)
            # new_entry here: PE filler during softmax
            neT = []
            for mt in range(2):
                sl = slice(mt * 128, (mt + 1) * 128)
                ps = ps_tile(f"ps_ne{mt}")
                mm_pair(W["w_ne"], sl, xT, ps)
                ne = act.tile([128, GC], FP32, tag=f"neT{mt}", name=f"neT{mt}")
                nc.scalar.activation(ne[:, :], ps[:, :], ACTF.Identity,
                                     bias=vbias("b_ne", mt), scale=1.0)
                neT.append(ne)
            s.update(attn_sb=attn_sb, neT=neT)

        def phase_L2(g):
            s = ST[g]
            memT, attn_sb = s["memT"], s["attn_sb"]
            MC = 2 * GC
            slab = big.tile([128, 5 * MC], BF, tag="pslab", name="pslab")
            for m in range(M):
                tmp = None
                if m >= 5:
                    tmp = big.tile([128, MC], BF, tag="ptmp", bufs=2,
                                   name=f"ptmp{m}")
                for t in range(2):
                    sl = slice(t * 128, (t + 1) * 128)
                    ps_v = pml_tile(f"ps_v{m}{t}")
                    mm_pair(W["w_v"], sl, memT[m], ps_v)
                    ps_er = pml_tile(f"ps_er{m}{t}")
                    nc.tensor.matmul(ps_er[:, :], amask_sb[m][t],
                                     attn_sb[:, :], start=True, stop=True)
                    er_sb = sml.tile([128, GC], BF, tag="er", bufs=1,
                                     name=f"er{m}{t}")
                    nc.scalar.copy(er_sb[:, :], ps_er[:, :])
                    dst = (slab[:, m * MC + t * GC:m * MC + (t + 1) * GC]
                           if m < 5 else tmp[:, t * GC:(t + 1) * GC])
                    nc.vector.tensor_tensor(dst, er_sb[:, :], ps_v[:, :],
                                            ALU.mult)
                if m >= 5:
                    c = (m - 5) * MC
                    nc.gpsimd.tensor_tensor(slab[:, c:c + MC],
                                            slab[:, c:c + MC],
                                            tmp[:, :], ALU.add)
            nc.vector.tensor_tensor(slab[:, 0:2 * MC], slab[:, 0:2 * MC],
                                    slab[:, 2 * MC:4 * MC], ALU.add)
            nc.vector.tensor_tensor(slab[:, 0:MC], slab[:, 0:MC],
                                    slab[:, MC:2 * MC], ALU.add)
            nc.vector.tensor_tensor(slab[:, 0:MC], slab[:, 0:MC],
                                    slab[:, 4 * MC:5 * MC], ALU.add)
            U_sb = []
            for t in range(2):
                u_t = act.tile([128, GC], BF, tag=f"U{t}", name=f"U{t}")
                nc.vector.tensor_copy(u_t[:, :], slab[:, t * GC:(t + 1) * GC])
                U_sb.append(u_t)
            s["U_sb"] = U_sb

        def ln_stats(x_b16_tiles, sq_tag):
            ps_s1 = ps_tile("ps_s1_" + sq_tag, [1, GC])
            s1b = sml.tile([1, GC], BF, tag="s1b", bufs=2,
                           name="s1b_" + sq_tag)
            for kt in range(2):
                nc.tensor.matmul(ps_s1[:, :], ones_128_1[:, :],
                                 x_b16_tiles[kt][:, :],
                                 start=(kt == 0), stop=(kt == 1))
            sq = [act.tile([128, GC], BF, tag=f"sq{kt}",
                           name=f"{sq_tag}{kt}") for kt in range(2)]
            for kt in range(2):
                nc.vector.tensor_tensor(sq[kt][:, :], x_b16_tiles[kt][:, :],
                                        x_b16_tiles[kt][:, :], ALU.mult)
            ps_s2 = ps_tile("ps_s2_" + sq_tag, [1, GC])
            for kt in range(2):
                nc.tensor.matmul(ps_s2[:, :], ones_128_1[:, :],
                                 sq[kt][:, :], start=(kt == 0), stop=(kt == 1))
            nc.vector.tensor_copy(s1b[:, :], ps_s1[:, :])
            s1sq = sml.tile([1, GC], FP32, tag="lnscr", bufs=2,
                            name="s1sq_" + sq_tag)
            nc.vector.tensor_tensor(s1sq[:, :], s1b[:, :], s1b[:, :],
                                    ALU.mult)
            var_t = sml.tile([1, GC], FP32, tag="lnscr", bufs=2,
                             name="var_" + sq_tag)
            nc.vector.scalar_tensor_tensor(var_t[:, :], ps_s2[:, :],
                                           float(D), s1sq[:, :],
                                           op0=ALU.mult, op1=ALU.subtract)
            sd = sml.tile([1, GC], FP32, tag="lnscr", bufs=2,
                          name="sd_" + sq_tag)
            nc.scalar.activation(sd[:, :], var_t[:, :], ACTF.Sqrt,
                                 bias=vecs_sb[0][0:1, VEC["eps_dd"]:
                                                 VEC["eps_dd"] + 1],
                                 scale=1.0)
            rc = sml.tile([1, GC], FP32, tag="lnscr", bufs=2,
                          name="rc_" + sq_tag)
            nc.vector.reciprocal_approx_fast(rc[:, :], sd[:, :])
            A_b = sml.tile([1, GC], BF, tag="A_b", name="A_" + sq_tag)
            with nc.allow_low_precision("LN scale bf16"):
                nc.vector.tensor_scalar(A_b[:, :], rc[:, :], float(D), None,
                                        op0=ALU.mult)
            B_b = sml.tile([1, GC], BF, tag="B_b", name="B_" + sq_tag)
            nc.vector.tensor_tensor(B_b[:, :], s1b[:, :], rc[:, :], ALU.mult)
            return A_b, B_b, s1b

        def phase_TAILA(g):
            s = ST[g]
            U_sb = s["U_sb"]
            ctx_b16 = []
            for mt in range(2):
                sl = slice(mt * 128, (mt + 1) * 128)
                ps = ps_tile(f"ps_ctx{mt}")
                for kt in range(2):
                    nc.tensor.matmul(ps[:, :], W["w_o"][kt][:, sl],
                                     U_sb[kt][:, :],
                                     start=(kt == 0), stop=(kt == 1))
                cb = act.tile([128, GC], BF, tag=f"ctxb{mt}", name=f"ctxb{mt}")
                nc.scalar.activation(cb[:, :], ps[:, :], ACTF.Identity,
                                     bias=vbias("bo_p", mt), scale=1.0)
                ctx_b16.append(cb)

            A1, B1, s1_b16 = ln_stats(ctx_b16, "sqc")
            ps_A1 = ps_tile("ps_A1rep")
            nc.tensor.matmul(ps_A1[:, :], ones_1_128[:, :], A1[:, :],
                             start=True, stop=True)
            A1rep = act.tile([128, GC], BF, tag="A1rep", name="A1rep")
            nc.scalar.copy(A1rep[:, :], ps_A1[:, :])

            ctxp_f, ctxp_b16 = [], []
            for mt in range(2):
                sl = slice(mt * 128, (mt + 1) * 128)
                ps = ps_tile(f"ps_cp{mt}")
                for kt in range(2):
                    nc.tensor.matmul(ps[:, :], W["w_ctx"][kt][:, sl],
                                     ctx_b16[kt][:, :],
                                     start=(kt == 0), stop=False)
                nc.tensor.matmul(ps[:, :], W["neg_gw"][:, sl], s1_b16[:, :],
                                 start=False, stop=True)
                tmul = act.tile([128, GC], BF, tag=f"cpm{mt}", name=f"cpm{mt}")
                nc.vector.tensor_tensor(tmul[:, :], A1rep[:, :], ps[:, :],
                                        ALU.mult)
                sgc = act.tile([128, GC], FP32, tag="fscr", bufs=3,
                               name=f"sgc{mt}")
                nc.scalar.activation(sgc[:, :], tmul[:, :], ACTF.Sigmoid,
                                     bias=vbias("bw_ctx2", mt), scale=2.0)
                cpf = act.tile([128, GC], FP32, tag=f"ctxp{mt}",
                               name=f"ctxp{mt}")
                nc.vector.tensor_scalar(cpf[:, :], sgc[:, :], 2.0, -1.0,
                                        op0=ALU.mult, op1=ALU.add)
                ctxp_f.append(cpf)
                cpb = act.tile([128, GC], BF, tag=f"ctxpb{mt}",
                               name=f"ctxpb{mt}")
                nc.vector.tensor_copy(cpb[:, :], cpf[:, :])
                ctxp_b16.append(cpb)
            s.update(ctxp_f=ctxp_f, ctxp_b16=ctxp_b16)

        def phase_TAILB(g):
            s = ST[g]
            gru_f, gru_b16 = s["gru_f"], s["gru_b16"]
            ctxp_f, ctxp_b16 = s["ctxp_f"], s["ctxp_b16"]
            neT = s["neT"]
            rows = slice(g * GC, (g + 1) * GC)
            alpha_f = []
            for mt in range(2):
                sl = slice(mt * 128, (mt + 1) * 128)
                ps = ps_tile(f"ps_al{mt}")
                for kt in range(2):
                    nc.tensor.matmul(ps[:, :], W["w_gate"][kt][:, sl],
                                     gru_b16[kt][:, :],
                                     start=(kt == 0), stop=False)
                for kt in range(2):
                    nc.tensor.matmul(ps[:, :], W["w_gate"][2 + kt][:, sl],
                                     ctxp_b16[kt][:, :],
                                     start=False, stop=(kt == 1))
                al = act.tile([128, GC], FP32, tag=f"alpha{mt}",
                              name=f"alpha{mt}")
                nc.scalar.activation(al[:, :], ps[:, :], ACTF.Sigmoid,
                                     bias=vbias("b_gate", mt), scale=1.0)
                alpha_f.append(al)

            blend_f, blend_b16 = [], []
            for mt in range(2):
                d2 = act.tile([128, GC], FP32, tag="fscr", bufs=3,
                              name=f"d2_{mt}")
                nc.vector.tensor_tensor(d2[:, :], ctxp_f[mt][:, :],
                                        gru_f[mt][:, :], ALU.subtract)
                e2 = act.tile([128, GC], FP32, tag="fscr", bufs=3,
                              name=f"e2_{mt}")
                nc.vector.tensor_tensor(e2[:, :], alpha_f[mt][:, :], d2[:, :],
                                        ALU.mult)
                bl = act.tile([128, GC], FP32, tag=f"blend{mt}",
                              name=f"blend{mt}")
                nc.vector.tensor_tensor(bl[:, :], gru_f[mt][:, :], e2[:, :],
                                        ALU.add)
                blend_f.append(bl)
                bb = act.tile([128, GC], BF, tag=f"blendb{mt}",
                              name=f"blendb{mt}")
                nc.vector.tensor_copy(bb[:, :], bl[:, :])
                blend_b16.append(bb)

            A2, B2, _s1b2 = ln_stats(blend_b16, "sqb")
            hcT = []
            for mt in range(2):
                sl = slice(mt * 128, (mt + 1) * 128)
                ps_Ag = ps_tile(f"ps_Ag{mt}")
                nc.tensor.matmul(ps_Ag[:, :], W["g_out_row"][:, sl], A2[:, :],
                                 start=True, stop=True)
                ps_Bg = ps_tile(f"ps_Bg{mt}")
                nc.tensor.matmul(ps_Bg[:, :], W["g_out_row"][:, sl], B2[:, :],
                                 start=True, stop=False)
                nc.tensor.matmul(ps_Bg[:, :], W["neg_beta_out_row"][:, sl],
                                 ones_row[:, :], start=False, stop=True)
                t3 = act.tile([128, GC], FP32, tag="fscr", bufs=3,
                              name=f"t3_{mt}")
                nc.vector.tensor_tensor(t3[:, :], blend_f[mt][:, :],
                                        ps_Ag[:, :], ALU.mult)
                hct = act.tile([128, GC], FP32, tag=f"hcT{mt}", name=f"hcT{mt}")
                nc.vector.tensor_tensor(hct[:, :], t3[:, :], ps_Bg[:, :],
                                        ALU.subtract)
                hcT.append(hct)

            hc_nat = sml.tile([128, NR4 * D], FP32, tag="hcnat", bufs=1,
                              name="hc_nat")
            ne_nat = sml.tile([128, NR4 * D], FP32, tag="nenat", bufs=1,
                              name="ne_nat")
            for r4 in range(NR4):
                ps = ps_tile(f"ps_otr{r4}", [128, D])
                for mt in range(2):
                    nc.tensor.transpose(ps[:, mt * 128:(mt + 1) * 128],
                                        hcT[mt][:, r4 * 128:(r4 + 1) * 128],
                                        ident)
                nc.vector.tensor_copy(hc_nat[:, r4 * D:(r4 + 1) * D],
                                      ps[:, :])
                ps2 = ps_tile(f"ps_otr2{r4}", [128, D])
                for mt in range(2):
                    nc.tensor.transpose(ps2[:, mt * 128:(mt + 1) * 128],
                                        neT[mt][:, r4 * 128:(r4 + 1) * 128],
                                        ident)
                nc.vector.tensor_copy(ne_nat[:, r4 * D:(r4 + 1) * D],
                                      ps2[:, :])
            nc.scalar.dma_start(
                hc_d[rows, :].rearrange("(a p) c -> p a c", p=128),
                hc_nat[:, :])
            nc.scalar.dma_start(
                nm_d[rows, (M - 1) * D:M * D].rearrange("(a p) c -> p a c",
                                                        p=128),
                ne_nat[:, :])

        # interleaved emission: group 1's PE-heavy phases fill group 0's
        # DVE/ACT-heavy tail gaps (and vice versa)
        preload(0)
        nc.sync.dma_start(kb_sb[:, :], kb_d[:, :])
        preload(1)

        # PE warm-up: keep the tensor engine busy (HAM at 8/8) while the
        # initial DMAs stream in; depends only on kf (first, small load).
        ps_w = ps_tile("ps_warm")
        for i in range(36):
            nc.tensor.transpose(ps_w[:, 0:128], ident, ident)
        warm_sink = sml.tile([1, 4], FP32, tag="wsink", name="warm_sink")
        nc.vector.tensor_copy(warm_sink[:, :], ps_w[0:1, 0:4])

        phase_T(0)
        phase_GRU(0)
        phase_L1(0)
        phase_SM(0)
        phase_L2(0)
        phase_T(1)
        phase_TAILA(0)
        phase_TAILB(0)
        phase_GRU(1)
        phase_L1(1)
        phase_SM(1)
        phase_L2(1)
        phase_TAILA(1)
        phase_TAILB(1)

    nc.compile()
    return nc


def _get_kernel():
    if "nc" not in _BUILD_CACHE:
        _BUILD_CACHE["nc"] = build_kernel()
    return _BUILD_CACHE["nc"]


def make_in_maps(inputs):
    w = _prep_weights(inputs)
    x = np.ascontiguousarray(np.asarray(inputs["inputs"], np.float32))
    h = np.ascontiguousarray(np.asarray(inputs["h_prev"], np.float32))
    mem = np.ascontiguousarray(np.asarray(inputs["memory_flat"], np.float32))
    in_maps = []
    for c in range(N_CORES):
        s = slice(c * BS, (c + 1) * BS)
        in_maps.append({"x": x[s], "h": h[s], "mem": mem[s],
                        "kb": w["kb"], "kf": w["kf"]})
    return in_maps


def kernel(**inputs):
    nc = _get_kernel()
    in_maps = make_in_maps(inputs)
    res = run_bass_kernel_spmd(nc, in_maps, core_ids=list(range(N_CORES)))
    h_corr = np.concatenate([res.results[c]["hcorr"] for c in range(N_CORES)],
                            axis=0)
    new_mem = np.concatenate([res.results[c]["newmem"] for c in range(N_CORES)],
                             axis=0)
    return h_corr, new_mem
